# revision 1
# baseline (speedup 1.0000x reference)
"""Trainium2 Bass kernel for nn_BiGNN_53772990546511.

Strategy (validated against the reference in numpy, global l2 rel err ~5e-7):
  - relu(elu(x)) == relu(x) exactly.
  - Location-node rows of h are identical from day 1 on, so the GAT for days
    2..4 degenerates to row algebra: every user with >=1 edge gets whl_j,
    everyone else (and all loc rows) gets mean(Wh_j); softmax of equal values
    is uniform.
  - Day-1 attention collapses to a 1024x1024 problem (users x locs) with a
    combined {0,1,2} multiplicity mask (mob+text; the two loc blocks of Wh are
    identical).  Softmax computed without max-subtraction (shift invariance;
    values are small), with a -10 bias so exp fits fp16.
  - exp(leaky(E)) == max(exp(E), exp(0.2E)); E = f1 (+) f2 outer sum is fused
    into the Exp activation (f1 broadcast as input, f2 as per-partition bias).
  - x_user is A_hat @ x_loc (A_hat = count-normalized day-0 adjacency built
    host-side from the integer edge list; no float compute on host).

Sharding: 8 cores = 4 batch pairs.  Both cores of a pair compute the full
(small) per-batch recurrence; each writes half of the output rows.  The
program is SPMD-uniform: odd cores receive user-axis-rotated (by 512) index
tensors so "local users 0..511" are global users 512..1023.
"""
import numpy as np

N_USER = 1024
N_LOC = 1024
DM = 256
HD = 256
B = 4
D = 5
E = 4096
ALPHA = 0.2
EXPBIAS = -10.0
P = 128
NCORES = 8

_CACHE = {}


# --------------------------------------------------------------------------
# Workarounds for this walrus build's 1-sync-wait-per-instruction limit.
# --------------------------------------------------------------------------
def _apply_tile_patch():
    import concourse.tile as tile
    from concourse.tile_sem_assignment import tick_to_sem

    if not getattr(tile.TileContext, "_drain_patched", False):
        def _patched(self, tick_clock, wait_clock):
            nc = self.nc
            gc = tick_clock.global_clock
            for proc, sem in self.sems.allocated().items():
                t = gc[proc]
                if t and t > 0:
                    nc.sync.nop().wait_op(sem, tick_to_sem(t, proc), "sem-ge")
            nc.sync.drain()
            nc.all_engine_barrier()
            popped = nc._tile_sem_poison_stack.pop()
            assert popped is self._sem_poison
            nc.clear_and_free_semaphores(list(self.sems.allocated().values()))
            nc.all_engine_barrier()

        tile.TileContext._drain_and_barrier = _patched
        tile.TileContext._drain_patched = True

    # Rewrite the BIR JSON just before compilation: hoist excess waits onto
    # injected same-engine NoOps (engines are in-order, so this preserves
    # semantics exactly).
    import json as _json
    import concourse.bass_utils as _bu
    import concourse.bass2jax as _b2j

    if not getattr(_bu, "_wait_split_patched", False):
        _orig_compile = _bu.compile_bir_kernel

        def _split_waits(bir_json):
            j = _json.loads(bir_json)
            nid = [0]
            for fn in j.get("functions", []):
                for bb in fn.get("blocks", []):
                    out = []
                    for inst in bb.get("instructions", []):
                        si = inst.get("sync_info") or {}
                        ow = si.get("on_wait") or []
                        if len(ow) > 1:
                            for w in ow[:-1]:
                                nid[0] += 1
                                out.append({
                                    "debug": inst.get("debug", 0),
                                    "engine": inst.get("engine", "SP"),
                                    "ins": [],
                                    "name": f"WSPL-{nid[0]}",
                                    "opcode": "NoOp",
                                    "outs": [],
                                    "sync_info": {"on_update": [],
                                                  "on_wait": [w]},
                                })
                            si["on_wait"] = [ow[-1]]
                        out.append(inst)
                    bb["instructions"] = out
            return _json.dumps(j).encode()

        def _patched_compile(bir_json, tmpdir, neff_name="file.neff"):
            return _orig_compile(_split_waits(bir_json), tmpdir,
                                 neff_name=neff_name)

        _bu.compile_bir_kernel = _patched_compile
        _b2j.compile_bir_kernel = _patched_compile
        _bu._wait_split_patched = True


def _build_nc():
    import concourse.bass as bass
    import concourse.tile as tile
    from concourse import mybir

    _apply_tile_patch()
    f32 = mybir.dt.float32
    f16 = mybir.dt.float16
    AF = mybir.ActivationFunctionType
    OP = mybir.AluOpType

    nc = bass.Bass()

    # ---------------- DRAM tensors ----------------
    d_xlocT = nc.dram_tensor("xlocT", [DM, N_LOC], f32, kind="ExternalInput")
    d_xloc16 = nc.dram_tensor("xloc16", [N_LOC, DM], f16, kind="ExternalInput")
    d_xlocf = nc.dram_tensor("xlocf", [N_LOC, DM], f32, kind="ExternalInput")
    d_AhatT = nc.dram_tensor("AhatT", [N_LOC, N_USER], f16, kind="ExternalInput")
    d_MT = nc.dram_tensor("MT", [N_LOC, N_USER], f16, kind="ExternalInput")
    d_W = nc.dram_tensor("W", [DM, HD], f32, kind="ExternalInput")
    d_W16 = nc.dram_tensor("W16", [DM, HD], f16, kind="ExternalInput")
    d_xlocT16 = nc.dram_tensor("xlocT16", [DM, N_LOC], f16, kind="ExternalInput")
    d_WT = nc.dram_tensor("WT", [HD, DM], f32, kind="ExternalInput")
    d_acol = nc.dram_tensor("acol", [2 * HD, 1], f32, kind="ExternalInput")
    d_nothas = nc.dram_tensor("nothas", [1, N_USER], f32, kind="ExternalInput")
    d_nothas16 = nc.dram_tensor("nothas16", [1, N_USER], f16, kind="ExternalInput")
    d_uw3 = nc.dram_tensor("uw3", [N_USER, 3], f32, kind="ExternalInput")
    d_hcol3 = nc.dram_tensor("hcol3", [3, 1], f32, kind="ExternalInput")
    d_cpair = nc.dram_tensor("cpair", [2, 1], f32, kind="ExternalInput")
    d_hasr16 = nc.dram_tensor("hasr16", [3, N_USER], f16, kind="ExternalInput")
    d_nothasr16 = nc.dram_tensor("nothasr16", [3, N_USER], f16,
                                 kind="ExternalInput")
    d_ner16 = nc.dram_tensor("ner16", [3, 2], f16, kind="ExternalInput")
    d_out = nc.dram_tensor("out", [D, 1536, HD], f32, kind="ExternalOutput")

    with tile.TileContext(nc) as tc:
        import contextlib
        with contextlib.ExitStack() as ctx:
            persist = ctx.enter_context(tc.tile_pool(name="persist", bufs=1))
            work = ctx.enter_context(tc.tile_pool(name="work", bufs=1))
            psA = ctx.enter_context(tc.tile_pool(name="psA", bufs=1, space="PSUM"))
            psB = ctx.enter_context(tc.tile_pool(name="psB", bufs=3, space="PSUM"))
            psR = ctx.enter_context(tc.tile_pool(name="psR", bufs=2, space="PSUM"))
            psC = ctx.enter_context(tc.tile_pool(name="psC", bufs=2, space="PSUM"))

            def big_load(dst, dram, t):
                src = dram.rearrange("(t p) u -> p t u", p=P)
                nc.sync.dma_start(
                    out=dst[:].rearrange("p (t u) -> p t u", t=t), in_=src)

            # ------------- load inputs (one DMA per tensor) -------------
            xlocT = persist.tile([P, 2 * N_LOC], f32, name="xlocT")
            big_load(xlocT, d_xlocT[:], 2)
            xloc16 = persist.tile([P, 8 * DM], f16, name="xloc16")
            big_load(xloc16, d_xloc16[:], 8)
            AhatT = persist.tile([P, 8 * N_USER], f16, name="AhatT")
            big_load(AhatT, d_AhatT[:], 8)
            MT = persist.tile([P, 8 * N_USER], f16, name="MTt")
            big_load(MT, d_MT[:], 8)
            Wsb = persist.tile([P, 2 * HD], f32, name="Wsb")
            big_load(Wsb, d_W[:], 2)
            WTsb = persist.tile([P, 2 * DM], f32, name="WTsb")
            big_load(WTsb, d_WT[:], 2)
            acol = persist.tile([P, 4], f32, name="acol")
            big_load(acol, d_acol[:], 4)
            nothas = persist.tile([1, N_USER], f32, name="nothas")
            nc.sync.dma_start(out=nothas[:], in_=d_nothas[:])
            nothas16 = persist.tile([1, N_USER], f16, name="nothas16")
            nc.sync.dma_start(out=nothas16[:], in_=d_nothas16[:])
            uw3 = persist.tile([P, 8 * 3], f32, name="uw3")
            big_load(uw3, d_uw3[:], 8)
            hcol3 = persist.tile([3, 1], f32, name="hcol3")
            nc.sync.dma_start(out=hcol3[:], in_=d_hcol3[:])
            cpair = persist.tile([2, 1], f32, name="cpair")
            nc.sync.dma_start(out=cpair[:], in_=d_cpair[:])
            hasr16 = persist.tile([1, 3 * N_USER], f16, name="hasr16")
            nc.sync.dma_start(
                out=hasr16[:].rearrange("p (t u) -> p t u", t=3),
                in_=d_hasr16[None, :, :])
            nothasr16 = persist.tile([1, 3 * N_USER], f16, name="nothasr16")
            nc.sync.dma_start(
                out=nothasr16[:].rearrange("p (t u) -> p t u", t=3),
                in_=d_nothasr16[None, :, :])
            ner16 = persist.tile([1, 6], f16, name="ner16")
            nc.sync.dma_start(out=ner16[:].rearrange("p (t u) -> p t u", t=3),
                              in_=d_ner16[None, :, :])

            W16 = persist.tile([P, 2 * HD], f16, name="W16")
            big_load(W16, d_W16[:], 2)
            xlocT16 = persist.tile([P, 2 * N_LOC], f16, name="xlocT16")
            big_load(xlocT16, d_xlocT16[:], 2)

            # constants
            ones32c = persist.tile([P, 1], f32, name="ones32c")
            nc.vector.memset(ones32c[:], 1.0)
            onesrow = persist.tile([1, P], f32, name="onesrow")
            nc.vector.memset(onesrow[:], 1.0)
            onesrow16 = persist.tile([1, P], f16, name="onesrow16")
            nc.vector.memset(onesrow16[:], 1.0)
            one11 = persist.tile([1, 1], f32, name="one11")
            nc.vector.memset(one11[:], 1.0)
            l3 = persist.tile([P, 3], f16, name="l3")
            nc.vector.memset(l3[:, 0:2], 0.0)
            nc.vector.memset(l3[:, 2:3], 1.0)

            def Wk(kt):
                return Wsb[:, kt * HD:(kt + 1) * HD]

            def W16k(kt):
                return W16[:, kt * HD:(kt + 1) * HD]

            def WTk(kt):
                return WTsb[:, kt * DM:(kt + 1) * DM]

            def xT(kt, sl):
                return xlocT[:, kt * N_LOC:(kt + 1) * N_LOC][:, sl]

            def xT16(kt, sl):
                return xlocT16[:, kt * N_LOC:(kt + 1) * N_LOC][:, sl]

            def xl16(lt):
                return xloc16[:, lt * DM:(lt + 1) * DM]

            def Ah(lt, sl):
                return AhatT[:, lt * N_USER:(lt + 1) * N_USER][:, sl]

            def Mk(lt):
                return MT[:, lt * N_USER:(lt + 1) * N_USER]

            def trans_row_to_col(row, dst2):
                """[1, 256] row -> two [128,1] col tiles (K=1 matmuls)."""
                for mt in range(2):
                    ps = psC.tile([P, 1], f32, name="pst", tag="ps1")
                    nc.tensor.matmul(ps[:], row[0:1, mt * P:(mt + 1) * P],
                                     one11[:], start=True, stop=True)
                    nc.vector.tensor_copy(dst2[mt][:], ps[:])

            # ---------------- phase 1: small matmuls ----------------
            wa1 = [persist.tile([P, 1], f32, name=f"wa1_{i}") for i in range(2)]
            wa2 = [persist.tile([P, 1], f32, name=f"wa2_{i}") for i in range(2)]
            for dst, ai in ((wa1, 0), (wa2, 2)):
                for mt in range(2):
                    ps = psC.tile([P, 1], f32, name="ps1", tag="ps1")
                    for kt in range(2):
                        nc.tensor.matmul(ps[:], WTk(kt)[:, mt * P:(mt + 1) * P],
                                         acol[:, ai + kt:ai + kt + 1],
                                         start=(kt == 0), stop=(kt == 1))
                    nc.vector.tensor_copy(dst[mt][:], ps[:])
            wa1_16 = [persist.tile([P, 1], f16, name=f"wa1h_{i}") for i in range(2)]
            wa2_16 = [persist.tile([P, 1], f16, name=f"wa2h_{i}") for i in range(2)]
            for kt in range(2):
                nc.vector.tensor_copy(wa1_16[kt][:], wa1[kt][:])
                nc.vector.tensor_copy(wa2_16[kt][:], wa2[kt][:])
            xw1 = [persist.tile([P, 1], f16, name=f"xw1_{i}") for i in range(8)]
            fb1 = [persist.tile([P, 1], f32, name=f"fb1_{i}") for i in range(8)]
            fb2 = [persist.tile([P, 1], f32, name=f"fb2_{i}") for i in range(8)]
            for lt in range(8):
                sl = slice(lt * P, (lt + 1) * P)
                ps = psC.tile([P, 1], f32, name="ps1", tag="ps1")
                for kt in range(2):
                    nc.tensor.matmul(ps[:], xT16(kt, sl), wa1_16[kt][:],
                                     start=(kt == 0), stop=(kt == 1))
                nc.vector.tensor_copy(xw1[lt][:], ps[:])
                ps = psC.tile([P, 1], f32, name="ps1", tag="ps1")
                for kt in range(2):
                    nc.tensor.matmul(ps[:], xT16(kt, sl), wa2_16[kt][:],
                                     start=(kt == 0), stop=(kt == 1))
                nc.vector.tensor_scalar(out=fb1[lt][:], in0=ps[:], scalar1=1.0,
                                        scalar2=EXPBIAS, op0=OP.mult, op1=OP.add)
                nc.vector.tensor_scalar(out=fb2[lt][:], in0=ps[:], scalar1=ALPHA,
                                        scalar2=EXPBIAS, op0=OP.mult, op1=OP.add)
            whext = [persist.tile([P, HD + 1], f16, name=f"whext{i}")
                     for i in range(8)]
            for lt in range(8):
                sl = slice(lt * P, (lt + 1) * P)
                ps = psB.tile([P, HD], f32, name="pb", tag="pb")
                for kt in range(2):
                    nc.tensor.matmul(ps[:], xT16(kt, sl), W16k(kt),
                                     start=(kt == 0), stop=(kt == 1))
                nc.vector.tensor_copy(whext[lt][:, 0:HD], ps[:])
                nc.vector.memset(whext[lt][:, HD:HD + 1], 1.0)

            # ---------------- phase 2: x_user ----------------
            xu_all = persist.tile([P, 8 * DM], f32, name="xu_all")
            for ut in range(8):
                ps = psB.tile([P, DM], f32, name="pb", tag="pb")
                for lt in range(8):
                    nc.tensor.matmul(ps[:], Ah(lt, slice(ut * P, (ut + 1) * P)),
                                     xl16(lt), start=(lt == 0), stop=(lt == 7))
                nc.vector.tensor_copy(xu_all[:, ut * DM:(ut + 1) * DM], ps[:])

            # ---------------- phase 3: means / day-0 ----------------
            # stack3 rows = [mwe; sxu; sxl] via two batched reduction chains
            ps3 = psB.tile([3, DM], f32, name="ps3", tag="pb")
            for ut in range(8):
                nc.tensor.matmul(ps3[:], uw3[:, ut * 3:(ut + 1) * 3],
                                 xu_all[:, ut * DM:(ut + 1) * DM],
                                 start=(ut == 0), stop=False)
            for lt in range(8):
                nc.tensor.matmul(ps3[:], l3[:], xl16(lt),
                                 start=False, stop=(lt == 7))
            stack3 = persist.tile([3, DM], f32, name="stack3")
            nc.vector.tensor_copy(stack3[:], ps3[:])
            mwe_row = stack3[0:1, :]
            mcol_in = [persist.tile([P, 1], f32, name=f"mcolin{i}") for i in range(2)]
            for mt in range(2):
                ps = psC.tile([P, 1], f32, name="ps1", tag="ps1")
                nc.tensor.matmul(ps[:], stack3[:, mt * P:(mt + 1) * P], hcol3[:],
                                 start=True, stop=True)
                nc.vector.tensor_copy(mcol_in[mt][:], ps[:])
            mw0_row = persist.tile([1, HD + 1], f32, name="mw0_row")
            ps = psR.tile([1, 512], f32, name="psr", tag="psr")
            for kt in range(2):
                nc.tensor.matmul(ps[:, 0:HD], mcol_in[kt][:], Wk(kt),
                                 start=(kt == 0), stop=(kt == 1))
            nc.vector.tensor_copy(mw0_row[:, 0:HD], ps[:, 0:HD])
            nc.vector.memset(mw0_row[:, HD:HD + 1], 1.0)
            mw0e16 = persist.tile([1, HD + 1], f16, name="mw0e16")
            nc.vector.tensor_copy(mw0e16[:], mw0_row[:])
            v1_row = persist.tile([1, HD], f32, name="v1_row")
            nc.scalar.activation(v1_row[:], mw0_row[:, 0:HD], AF.Relu)
            v1_16 = persist.tile([1, HD], f16, name="v1_16")
            nc.vector.tensor_copy(v1_16[:], v1_row[:])
            v1_col = [persist.tile([P, 1], f32, name=f"v1c{i}") for i in range(2)]
            trans_row_to_col(v1_row, v1_col)
            mwe_col = [persist.tile([P, 1], f32, name=f"mwec{i}") for i in range(2)]
            trans_row_to_col(stack3[0:1, 0:DM], mwe_col)
            s1_16 = persist.tile([1, 1], f16, name="s1_16")
            ps = psR.tile([1, 512], f32, name="psr", tag="psr")
            for kt in range(2):
                nc.tensor.matmul(ps[:, 0:1], wa1[kt][:], mwe_col[kt][:],
                                 start=(kt == 0), stop=(kt == 1))
            nc.vector.tensor_copy(s1_16[:], ps[:, 0:1])
            for ut in range(4):
                ps = psB.tile([P, DM], f32, name="pb", tag="pb")
                nc.tensor.matmul(ps[:], nothas[0:1, ut * P:(ut + 1) * P],
                                 mwe_row, start=True, stop=True)
                sl = slice(ut * DM, (ut + 1) * DM)
                nc.vector.tensor_tensor(out=xu_all[:, sl], in0=xu_all[:, sl],
                                        in1=ps[:], op=OP.add)
            nc.sync.dma_start(
                out=d_out[0, 0:512, :].rearrange("(t p) h -> p t h", p=P),
                in_=xu_all[:, 0:4 * DM].rearrange("p (t h) -> p t h", t=4))
            nc.sync.dma_start(out=d_out[0, 512:1536, :], in_=d_xlocf[:, :])

            # ---------------- phase 4: f1 row ----------------
            f1row = persist.tile([1, N_USER], f32, name="f1row")
            for ch in range(2):
                csl = slice(ch * 512, (ch + 1) * 512)
                ps = psR.tile([1, 512], f32, name="psr", tag="psr")
                for lt in range(8):
                    nc.tensor.matmul(ps[:], xw1[lt][:], Ah(lt, csl),
                                     start=(lt == 0), stop=False)
                nc.tensor.matmul(ps[:], s1_16[:], nothas16[:, csl],
                                 start=False, stop=True)
                nc.vector.tensor_copy(f1row[0:1, csl], ps[:])

            # ---------------- phase 5: e-stage ----------------
            F1B = persist.tile([P, N_USER], f32, name="F1B")
            for ch in range(2):
                csl = slice(ch * 512, (ch + 1) * 512)
                ps = psA.tile([P, 512], f32, name="psa", tag="psa")
                nc.tensor.matmul(ps[:], onesrow[:], f1row[0:1, csl],
                                 start=True, stop=True)
                nc.vector.tensor_copy(F1B[:, csl], ps[:])
            PT = persist.tile([P, 8 * N_USER], f16, name="PT")
            for lt in range(8):
                base = lt * N_USER
                for ch in range(2):
                    csl = slice(ch * 512, (ch + 1) * 512)
                    osl = slice(base + ch * 512, base + (ch + 1) * 512)
                    x1 = work.tile([P, 512], f16, name="x1", tag="x1", bufs=3)
                    x2 = work.tile([P, 512], f16, name="x2", tag="x2", bufs=3)
                    nc.scalar.activation(x1[:], F1B[:, csl], AF.Exp,
                                         bias=fb1[lt][:])
                    nc.scalar.activation(x2[:], F1B[:, csl], AF.Exp,
                                         bias=fb2[lt][:], scale=ALPHA)
                    nc.vector.tensor_tensor(out=PT[:, osl], in0=x1[:],
                                            in1=x2[:], op=OP.max)
                nc.vector.tensor_tensor(
                    out=PT[:, base:base + N_USER], in0=PT[:, base:base + N_USER],
                    in1=Mk(lt), op=OP.mult)

            # ---------------- phase 6: day-1 attention ----------------
            h1u_all = persist.tile([P, 8 * DM], f32, name="h1u_all")
            for ut in range(8):
                ps = psB.tile([P, HD + 1], f32, name="pb", tag="pb")
                for lt in range(8):
                    nc.tensor.matmul(
                        ps[:],
                        PT[:, lt * N_USER + ut * P:lt * N_USER + (ut + 1) * P],
                        whext[lt][:], start=(lt == 0), stop=False)
                nc.tensor.matmul(ps[:], nothas16[:, ut * P:(ut + 1) * P],
                                 mw0e16[:], start=False, stop=True)
                zr = work.tile([P, 1], f32, name="zr", tag="zr", bufs=3)
                nc.vector.reciprocal(zr[:], ps[:, HD:HD + 1])
                nc.vector.tensor_scalar(out=h1u_all[:, ut * DM:(ut + 1) * DM],
                                        in0=ps[:, 0:HD], scalar1=zr[:],
                                        scalar2=0.0, op0=OP.mult, op1=OP.max)
            nc.sync.dma_start(
                out=d_out[1, 0:512, :].rearrange("(t p) h -> p t h", p=P),
                in_=h1u_all[:, 0:4 * DM].rearrange("p (t h) -> p t h", t=4))

            def bcast_loc(day, vrow16):
                ps = psB.tile([P, HD], f32, name="pb", tag="pb")
                nc.tensor.matmul(ps[:], onesrow16[:], vrow16[:],
                                 start=True, stop=True)
                vst = work.tile([P, HD], f32, name="vst", tag="vst", bufs=2)
                nc.vector.tensor_copy(vst[:], ps[:])
                ap = vst[:]
                src = bass.AP(tensor=ap.tensor, offset=ap.offset,
                              ap=[list(ap.ap[0]), [0, 8], list(ap.ap[1])])
                dst = d_out[day, 512:1536, :].rearrange("(t p) h -> p t h", p=P)
                nc.sync.dma_start(out=dst, in_=src)

            bcast_loc(1, v1_16)

            # ---------------- phase 7: days 2..4 ----------------
            scol = [persist.tile([P, 1], f32, name=f"scol0_{i}") for i in range(2)]
            for mt in range(2):
                ps = psC.tile([P, 1], f32, name="ps1", tag="ps1")
                for ut in range(8):
                    nc.tensor.matmul(
                        ps[:], h1u_all[:, ut * DM + mt * P:ut * DM + (mt + 1) * P],
                        ones32c[:], start=(ut == 0), stop=(ut == 7))
                nc.vector.tensor_copy(scol[mt][:], ps[:])

            vcol_cur, scol_cur = v1_col, scol
            outu = persist.tile([P, 4 * HD], f32, name="outu")
            for day in (2, 3, 4):
                dd = day - 2
                vs2 = [work.tile([P, 2], f32, name=f"vs{day}_{kt}",
                                 tag=f"vs{day}_{kt}") for kt in range(2)]
                for kt in range(2):
                    nc.vector.tensor_copy(vs2[kt][:, 0:1], vcol_cur[kt][:])
                    nc.vector.tensor_copy(vs2[kt][:, 1:2], scol_cur[kt][:])
                ps2 = psB.tile([2, HD], f32, name="pb", tag="pb")
                for kt in range(2):
                    nc.tensor.matmul(ps2[:], vs2[kt][:], Wk(kt),
                                     start=(kt == 0), stop=(kt == 1))
                stk = work.tile([2, HD], f32, name=f"stk{day}", tag=f"stk{day}")
                nc.vector.tensor_copy(stk[:], ps2[:])
                psm = psR.tile([1, 512], f32, name="psr", tag="psr")
                nc.tensor.matmul(psm[:, 0:HD], cpair[:], stk[:],
                                 start=True, stop=True)
                m_row = work.tile([1, HD], f32, name=f"m{day}", tag=f"m{day}")
                nc.vector.tensor_copy(m_row[:], psm[:, 0:HD])
                r1_16 = work.tile([1, HD], f16, name=f"r1f{day}", tag=f"r1f{day}")
                nc.scalar.activation(r1_16[:], stk[0:1, :], AF.Relu)
                vn_row = work.tile([1, HD], f32, name=f"vn{day}", tag=f"vn{day}")
                nc.scalar.activation(vn_row[:], m_row[:], AF.Relu)
                vn_16 = work.tile([1, HD], f16, name=f"vn16{day}",
                                  tag=f"vn16{day}")
                nc.vector.tensor_copy(vn_16[:], vn_row[:])
                for ut in range(4):
                    ps = psB.tile([P, HD], f32, name="pb", tag="pb")
                    usl = slice(dd * N_USER + ut * P, dd * N_USER + (ut + 1) * P)
                    nc.tensor.matmul(ps[:], hasr16[:, usl], r1_16[:],
                                     start=True, stop=False)
                    nc.tensor.matmul(ps[:], nothasr16[:, usl], vn_16[:],
                                     start=False, stop=True)
                    nc.vector.tensor_copy(outu[:, ut * HD:(ut + 1) * HD], ps[:])
                nc.sync.dma_start(
                    out=d_out[day, 0:512, :].rearrange("(t p) h -> p t h", p=P),
                    in_=outu[:, 0:4 * HD].rearrange("p (t h) -> p t h", t=4))
                bcast_loc(day, vn_16)
                if day < 4:
                    vcol_n = [work.tile([P, 1], f32, name=f"vc{day}_{k}",
                                        tag=f"vc{day}_{k}") for k in range(2)]
                    trans_row_to_col(vn_row, vcol_n)
                    scol_n = [work.tile([P, 1], f32, name=f"sn{day}_{k}",
                                        tag=f"sn{day}_{k}") for k in range(2)]
                    for mt in range(2):
                        ps = psC.tile([P, 1], f32, name="ps1", tag="ps1")
                        msl = slice(mt * P, (mt + 1) * P)
                        nc.tensor.matmul(ps[:], r1_16[0:1, msl],
                                         ner16[:, dd * 2:dd * 2 + 1],
                                         start=True, stop=False)
                        nc.tensor.matmul(ps[:], vn_16[0:1, msl],
                                         ner16[:, dd * 2 + 1:dd * 2 + 2],
                                         start=False, stop=True)
                        nc.vector.tensor_copy(scol_n[mt][:], ps[:])
                    vcol_cur, scol_cur = vcol_n, scol_n

    return nc


def _host_prep(x_loc, mob_links, text_links, W, a):
    """Index-only preprocessing -> per-core input maps."""
    x_loc = np.ascontiguousarray(x_loc, np.float32)
    W = np.ascontiguousarray(W, np.float32)
    a = np.ascontiguousarray(a, np.float32)
    mob = np.asarray(mob_links)
    text = np.asarray(text_links)

    shared = {
        "xlocT": np.ascontiguousarray(x_loc.T),
        "xlocT16": np.ascontiguousarray(x_loc.T).astype(np.float16),
        "W16": W.astype(np.float16),
        "xloc16": x_loc.astype(np.float16),
        "xlocf": x_loc,
        "W": W,
        "WT": np.ascontiguousarray(W.T),
        "acol": a.reshape(2 * HD, 1),
        "cpair": np.array([[2048.0 / 3072.0], [1.0 / 3072.0]], np.float32),
    }

    in_maps = []
    for c in range(NCORES):
        b, r = c // 2, c % 2
        rot = r * 512
        u0 = np.concatenate([mob[b, 0, :, 0], text[b, 0, :, 0]]).astype(np.int64)
        l0 = np.concatenate([mob[b, 0, :, 1], text[b, 0, :, 1]]).astype(np.int64)
        cnt = np.bincount(u0, minlength=N_USER).astype(np.float32)
        A = np.zeros((N_USER, N_LOC), np.float32)
        np.add.at(A, (u0, l0), 1.0)
        Ahat = A / np.maximum(cnt, 1.0)[:, None]
        Mb = np.zeros((N_USER, N_LOC), np.float32)
        Tb = np.zeros((N_USER, N_LOC), np.float32)
        Mb[mob[b, 0, :, 0], mob[b, 0, :, 1]] = 1.0
        Tb[text[b, 0, :, 0], text[b, 0, :, 1]] = 1.0
        M = Mb + Tb
        has0 = (cnt > 0).astype(np.float32)
        n_with = max(float(has0.sum()), 1.0)
        nh_cnt = float(N_USER) - float(has0.sum())

        def rollu(x, axis=0):
            return np.roll(x, -rot, axis=axis)

        hasE = np.zeros((3, N_USER), np.float32)
        for dd in range(3):
            us = np.concatenate([mob[b, dd + 1, :, 0], text[b, dd + 1, :, 0]])
            hasE[dd, us] = 1.0
        hasrow = np.stack([rollu(hasE[dd]) for dd in range(3)]).astype(np.float32)
        nerow = np.stack([np.array([hasE[dd].sum(), N_USER - hasE[dd].sum()],
                                   np.float32) for dd in range(3)])
        nothas = rollu(1.0 - has0)[None, :].astype(np.float32)
        hw = (rollu(has0) / n_with).astype(np.float32)
        # stack3 row order [mwe; sxu; sxl] -> uw3 cols [hw, 1, 0]
        uw3 = np.stack([hw, np.ones(N_USER, np.float32),
                        np.zeros(N_USER, np.float32)], axis=1)
        m = dict(shared)
        m.update({
            "AhatT": np.ascontiguousarray(rollu(Ahat, 0).T).astype(np.float16),
            "MT": np.ascontiguousarray(rollu(M, 0).T).astype(np.float16),
            "nothas": nothas,
            "nothas16": nothas.astype(np.float16),
            "uw3": np.ascontiguousarray(uw3),
            "hcol3": np.array([[nh_cnt / 3072.0], [1.0 / 3072.0],
                               [2.0 / 3072.0]], np.float32),
            "hasr16": hasrow.astype(np.float16),
            "nothasr16": (1.0 - hasrow).astype(np.float16),
            "ner16": nerow.astype(np.float16),
        })
        in_maps.append(m)
    return in_maps


def kernel(**inputs):
    from concourse.bass_utils import run_bass_kernel_spmd

    if "nc" not in _CACHE:
        _CACHE["nc"] = _build_nc()
    nc = _CACHE["nc"]

    in_maps = _host_prep(inputs["x_loc"], inputs["mob_links"],
                         inputs["text_links"], inputs["W"], inputs["a"])
    res = run_bass_kernel_spmd(nc, in_maps, core_ids=list(range(NCORES)))

    out = np.zeros((B, D, N_USER + 2 * N_LOC, HD), np.float32)
    for c in range(NCORES):
        b, r = c // 2, c % 2
        o = res.results[c]["out"]
        out[b, :, r * 512:(r + 1) * 512, :] = o[:, 0:512, :]
        out[b, :, N_USER + r * N_LOC:N_USER + (r + 1) * N_LOC, :] = o[:, 512:1536, :]
    return out



# revision 12
# speedup vs baseline: 1.1725x; 1.1725x over previous
"""Trainium2 Bass kernel for nn_BiGNN_53772990546511.

Math (validated vs reference in mathcheck.py, global l2 rel ~2.5e-4):
  - relu(elu(x)) == relu(x).
  - Day-1 attention collapses to users x locs with multiplicity mask M.
    Softmax is invariant to per-column scaling, so
      exp(leaky(f1_u + f2_l)) ~ max(g_l, t_u) scaled per-row by c_l,
    with g = exp(0.8 f2 + B), t = exp(-0.8 f1 + B), c = exp(0.2 f2 + G).
    The c_l row factor is folded into the whext rows (including the ones
    column), so PT = max(g_l, t_u) * M needs only 2 elementwise ops/tile.
  - Days 2..4 are rank-2 row algebra kept in column form on device (no
    transposes); only 10 row-vectors leave the device and the host
    broadcasts them into the full output (pure gather/unshard).
  - All global sums over users (mwe, mean h0) are rank-1: host prepares
    index-derived weight vectors (w_mwe, w_h0) so phase-2 only computes
    the 512 own-user rows.
  - Device writes per core: 0.5MB (day0+day1 own user halves, f16) +
    rows; everything else is broadcast host-side from row vectors.

Sharding: 8 cores = 4 batches x 2 user-halves.  Odd cores get index
tensors rotated by 512 along users so the program is SPMD-uniform.
"""
import numpy as np

N_USER = 1024
N_LOC = 1024
DM = 256
HD = 256
B = 4
D = 5
E = 4096
ALPHA = 0.2
BETA = -1.0
GAMMA = -0.5
P = 128
NCORES = 8

_CACHE = {}


# --------------------------------------------------------------------------
# Workarounds for this walrus build's 1-sync-wait-per-instruction limit.
# --------------------------------------------------------------------------
def _apply_tile_patch():
    import concourse.tile as tile
    from concourse.tile_sem_assignment import tick_to_sem

    if not getattr(tile.TileContext, "_drain_patched", False):
        def _patched(self, tick_clock, wait_clock):
            nc = self.nc
            gc = tick_clock.global_clock
            for proc, sem in self.sems.allocated().items():
                t = gc[proc]
                if t and t > 0:
                    nc.sync.nop().wait_op(sem, tick_to_sem(t, proc), "sem-ge")
            nc.sync.drain()
            nc.all_engine_barrier()
            popped = nc._tile_sem_poison_stack.pop()
            assert popped is self._sem_poison
            nc.clear_and_free_semaphores(list(self.sems.allocated().values()))
            nc.all_engine_barrier()

        tile.TileContext._drain_and_barrier = _patched
        tile.TileContext._drain_patched = True

    import json as _json
    import concourse.bass_utils as _bu
    import concourse.bass2jax as _b2j

    if not getattr(_bu, "_wait_split_patched", False):
        _orig_compile = _bu.compile_bir_kernel

        def _split_waits(bir_json):
            j = _json.loads(bir_json)
            nid = [0]
            for fn in j.get("functions", []):
                for bb in fn.get("blocks", []):
                    out = []
                    for inst in bb.get("instructions", []):
                        si = inst.get("sync_info") or {}
                        ow = si.get("on_wait") or []
                        if len(ow) > 1:
                            for w in ow[:-1]:
                                nid[0] += 1
                                out.append({
                                    "debug": inst.get("debug", 0),
                                    "engine": inst.get("engine", "SP"),
                                    "ins": [],
                                    "name": f"WSPL-{nid[0]}",
                                    "opcode": "NoOp",
                                    "outs": [],
                                    "sync_info": {"on_update": [],
                                                  "on_wait": [w]},
                                })
                            si["on_wait"] = [ow[-1]]
                        out.append(inst)
                    bb["instructions"] = out
            return _json.dumps(j).encode()

        def _patched_compile(bir_json, tmpdir, neff_name="file.neff"):
            return _orig_compile(_split_waits(bir_json), tmpdir,
                                 neff_name=neff_name)

        _bu.compile_bir_kernel = _patched_compile
        _b2j.compile_bir_kernel = _patched_compile
        _bu._wait_split_patched = True


def _build_nc():
    import contextlib
    import concourse.bass as bass
    import concourse.tile as tile
    from concourse import mybir

    _apply_tile_patch()
    f32 = mybir.dt.float32
    f16 = mybir.dt.float16
    f8 = mybir.dt.float8e4
    AF = mybir.ActivationFunctionType
    OP = mybir.AluOpType

    nc = bass.Bass()

    # ---------------- DRAM tensors ----------------
    d_W16 = nc.dram_tensor("W16", [DM, HD], f16, kind="ExternalInput")
    d_WT16 = nc.dram_tensor("WT16", [HD, DM], f16, kind="ExternalInput")
    d_acol2 = nc.dram_tensor("acol2", [HD, 2], f16, kind="ExternalInput")
    d_wcols = nc.dram_tensor("wcols", [N_LOC, 2], f16, kind="ExternalInput")
    d_nothas = nc.dram_tensor("nothas16", [1, N_USER], f16, kind="ExternalInput")
    d_necol = nc.dram_tensor("necol", [P, 6], f32, kind="ExternalInput")
    d_xlocT = nc.dram_tensor("xlocT16", [DM, N_LOC], f16, kind="ExternalInput")
    d_xloc = nc.dram_tensor("xloc16", [N_LOC, DM], f16, kind="ExternalInput")
    d_A8 = nc.dram_tensor("Ahat8", [N_LOC, N_USER], f8, kind="ExternalInput")
    d_MT = nc.dram_tensor("MT16", [N_LOC, N_USER], f16, kind="ExternalInput")
    d_o01 = nc.dram_tensor("o01", [2, 512, HD], f16, kind="ExternalOutput")
    d_rows = nc.dram_tensor("rows", [P, 14], f32, kind="ExternalOutput")

    with tile.TileContext(nc) as tc:
        with contextlib.ExitStack() as ctx:
            persist = ctx.enter_context(tc.tile_pool(name="persist", bufs=1))
            work = ctx.enter_context(tc.tile_pool(name="work", bufs=1))
            psBig = ctx.enter_context(tc.tile_pool(name="psBig", bufs=2, space="PSUM"))
            ps6 = ctx.enter_context(tc.tile_pool(name="ps6", bufs=4, space="PSUM"))
            psSm = psBig
            psCol = psBig

            # ------------- input loads, in arrival-priority order -------------
            def big_load(dst, dram, t):
                src = dram.rearrange("(t p) u -> p t u", p=P)
                nc.sync.dma_start(
                    out=dst[:].rearrange("p (t u) -> p t u", t=t), in_=src)

            Wsb = persist.tile([P, 2 * HD], f16, name="Wsb")
            big_load(Wsb, d_W16[:], 2)
            WTsb = persist.tile([P, 2 * DM], f16, name="WTsb")
            big_load(WTsb, d_WT16[:], 2)
            acol = persist.tile([P, 4], f16, name="acol")
            big_load(acol, d_acol2[:], 2)
            wcol = persist.tile([P, 16], f16, name="wcol")
            big_load(wcol, d_wcols[:], 8)
            nothas = persist.tile([1, N_USER], f16, name="nothas")
            nc.sync.dma_start(out=nothas[:], in_=d_nothas[:])
            necol = persist.tile([P, 6], f32, name="necol")
            nc.sync.dma_start(out=necol[:], in_=d_necol[:])
            xlocT = persist.tile([P, 2 * N_LOC], f16, name="xlocT")
            big_load(xlocT, d_xlocT[:], 2)
            xloc = persist.tile([P, 8 * DM], f16, name="xloc")
            big_load(xloc, d_xloc[:], 8)
            A8 = persist.tile([P, 8 * N_USER], f8, name="A8")
            big_load(A8, d_A8[:], 8)
            MT = persist.tile([P, 8 * N_USER], f16, name="MT")
            for lt in range(8):
                nc.sync.dma_start(
                    out=MT[:].rearrange("p (t u) -> p t u", t=8)[:, lt:lt + 1, :],
                    in_=d_MT.rearrange("(t p) u -> p t u", p=P)[:, lt:lt + 1, :])

            def Wk(kt):
                return Wsb[:, kt * HD:(kt + 1) * HD]

            def WTk(kt):
                return WTsb[:, kt * DM:(kt + 1) * DM]

            def xT(kt, sl):
                return xlocT[:, kt * N_LOC:(kt + 1) * N_LOC][:, sl]

            def xl(lt):
                return xloc[:, lt * DM:(lt + 1) * DM]

            def Ah(lt, sl):
                return A8[:, lt * N_USER:(lt + 1) * N_USER][:, sl]

            def Mk(lt):
                return MT[:, lt * N_USER:(lt + 1) * N_USER]

            # constants
            ones16r = persist.tile([1, P], f16, name="ones16r")
            nc.vector.memset(ones16r[:], 1.0)
            one11 = persist.tile([1, 1], f16, name="one11")
            nc.vector.memset(one11[:], 1.0)
            ones16c = persist.tile([P, 1], f16, name="ones16c")
            nc.vector.memset(ones16c[:], 1.0)
            cBETA = persist.tile([P, 1], f32, name="cBETA")
            nc.vector.memset(cBETA[:], BETA)
            cGAMMA = persist.tile([P, 1], f32, name="cGAMMA")
            nc.vector.memset(cGAMMA[:], GAMMA)

            # persistent intermediates
            wa12 = [persist.tile([P, 2], f16, name=f"wa12_{kt}") for kt in range(2)]
            xw1c = persist.tile([P, 8], f16, name="xw1c")
            xw2c = persist.tile([P, 8], f32, name="xw2c")
            g32 = persist.tile([P, 8], f32, name="g32")
            c32 = persist.tile([P, 8], f32, name="c32")
            whext = [persist.tile([P, HD + 1], f16, name=f"whext{lt}")
                     for lt in range(8)]
            mh0row = persist.tile([1, DM], f16, name="mh0row")
            mwe16 = persist.tile([1, DM], f16, name="mwe16")
            mh0c = persist.tile([P, 2], f16, name="mh0c")
            mw0e = persist.tile([1, HD + 1], f16, name="mw0e")
            t16 = persist.tile([1, N_USER], f16, name="t16")
            Tb16 = persist.tile([P, N_USER], f16, name="Tb16")
            PT = persist.tile([P, 8 * N_USER], f16, name="PT")
            day0 = persist.tile([P, 4 * HD], f16, name="day0")
            h1u = persist.tile([P, 8 * HD], f16, name="h1u")
            rows_out = persist.tile([P, 14], f32, name="rows_out")
            rcol = [persist.tile([P, 1], f32, name=f"rcol{ut}") for ut in range(8)]

            # ---------------- wa12 = [W a1 | W a2] as cols ----------------
            for kt in range(2):
                ps = psSm.tile([P, 2], f32, name="psm", tag="sm", bufs=1)
                for ht in range(2):
                    nc.tensor.matmul(ps[:], WTk(ht)[:, kt * P:(kt + 1) * P],
                                     acol[:, ht * 2:ht * 2 + 2],
                                     start=(ht == 0), stop=(ht == 1))
                nc.scalar.activation(wa12[kt][:], ps[:], AF.Copy)

            # ---------------- xw12 cols per loc tile ----------------
            for lt in range(8):
                ps = psSm.tile([P, 2], f32, name="psm", tag="sm", bufs=1)
                for kt in range(2):
                    nc.tensor.matmul(ps[:], xT(kt, slice(lt * P, (lt + 1) * P)),
                                     wa12[kt][:], start=(kt == 0), stop=(kt == 1))
                nc.vector.tensor_copy(xw1c[:, lt:lt + 1], ps[:, 0:1])
                nc.vector.tensor_copy(xw2c[:, lt:lt + 1], ps[:, 1:2])
            nc.scalar.activation(g32[:], xw2c[:], AF.Exp, scale=0.8, bias=cBETA[:])
            nc.scalar.activation(c32[:], xw2c[:], AF.Exp, scale=0.2, bias=cGAMMA[:])

            # ---------------- whext_c = c_l * [Wh_l | 1] ----------------
            for lt in range(8):
                ps = psBig.tile([P, HD], f32, name="pwh", tag="big")
                for kt in range(2):
                    nc.tensor.matmul(ps[:], xT(kt, slice(lt * P, (lt + 1) * P)),
                                     Wk(kt), start=(kt == 0), stop=(kt == 1))
                nc.scalar.activation(whext[lt][:, 0:HD], ps[:], AF.Copy,
                                     scale=c32[:, lt:lt + 1])
                nc.vector.tensor_copy(whext[lt][:, HD:HD + 1], c32[:, lt:lt + 1])

            # ---------------- mh0 / mwe rows ----------------
            pmr = psBig.tile([1, DM], f32, name="pmr", tag="big")
            for lt in range(8):
                nc.tensor.matmul(pmr[:], wcol[:, lt * 2:lt * 2 + 1], xl(lt),
                                 start=(lt == 0), stop=(lt == 7))
            nc.scalar.activation(mh0row[:], pmr[:], AF.Copy)
            pmr2 = psBig.tile([1, DM], f32, name="pmr2", tag="big")
            for lt in range(8):
                nc.tensor.matmul(pmr2[:], wcol[:, lt * 2 + 1:lt * 2 + 2], xl(lt),
                                 start=(lt == 0), stop=(lt == 7))
            nc.scalar.activation(mwe16[:], pmr2[:], AF.Copy)
            for mt in range(2):
                ps = psSm.tile([P, 1], f32, name="psm", tag="sm", bufs=1)
                nc.tensor.matmul(ps[:], mh0row[0:1, mt * P:(mt + 1) * P],
                                 one11[:], start=True, stop=True)
                nc.vector.tensor_copy(mh0c[:, mt:mt + 1], ps[:])
            pmw = psBig.tile([1, HD], f32, name="pmw", tag="big")
            for kt in range(2):
                nc.tensor.matmul(pmw[:], mh0c[:, kt:kt + 1], Wk(kt),
                                 start=(kt == 0), stop=(kt == 1))
            nc.scalar.activation(mw0e[:, 0:HD], pmw[:], AF.Copy)
            nc.vector.memset(mw0e[:, HD:HD + 1], 1.0)

            # mw0 cols -> v1 (day-1 loc row) and vs-chain init
            vs1 = [work.tile([P, 3], f16, name=f"vs1_{kt}", tag=f"vs1_{kt}")
                   for kt in range(2)]
            for mt in range(2):
                ps = psCol.tile([P, 1], f32, name="pcol", tag="col", bufs=1)
                for kt in range(2):
                    nc.tensor.matmul(ps[:], Wk(kt)[:, mt * P:(mt + 1) * P],
                                     mh0c[:, kt:kt + 1],
                                     start=(kt == 0), stop=(kt == 1))
                nc.scalar.activation(rows_out[:, mt:mt + 1], ps[:], AF.Relu)
                nc.scalar.activation(vs1[mt][:, 0:1], ps[:], AF.Relu,
                                     scale=2048.0 / 3072.0)

            # ---------------- f1row -> t -> Tb ----------------
            for ch in range(2):
                csl = slice(ch * 512, (ch + 1) * 512)
                pf = psBig.tile([1, 512], f32, name="pf1", tag="big")
                for lt in range(8):
                    nc.tensor.matmul(pf[:], xw1c[:, lt:lt + 1], Ah(lt, csl),
                                     start=(lt == 0), stop=(lt == 7))
                nc.scalar.activation(t16[0:1, csl], pf[:], AF.Exp,
                                     scale=-0.8, bias=cBETA[0:1, :])
                ptb = psBig.tile([P, 512], f32, name="ptb", tag="big")
                nc.tensor.matmul(ptb[:], ones16r[:], t16[0:1, csl],
                                 start=True, stop=True)
                nc.scalar.activation(Tb16[:, csl], ptb[:], AF.Copy)

            # ---------------- phase 2: x_user own half ----------------
            for ut in range(4):
                ps = psBig.tile([P, DM], f32, name="px", tag="big")
                for lt in range(8):
                    nc.tensor.matmul(ps[:], Ah(lt, slice(ut * P, (ut + 1) * P)),
                                     xl(lt), start=(lt == 0), stop=False)
                nc.tensor.matmul(ps[:], nothas[0:1, ut * P:(ut + 1) * P],
                                 mwe16[:], start=False, stop=True)
                nc.scalar.activation(day0[:, ut * HD:(ut + 1) * HD], ps[:], AF.Copy)
            nc.scalar.dma_start(
                out=d_o01[0].rearrange("(t p) h -> p t h", p=P),
                in_=day0[:].rearrange("p (t h) -> p t h", t=4))

            # ---------------- PT + phase 6 (interleaved over loc tiles) -----
            for half in range(2):
                tiles = [ps6.tile([P, HD + 1], f32, name=f"p6_{half}_{i}",
                                  tag="p6") for i in range(4)]
                for lt in range(8):
                    if half == 0:
                        psl = slice(lt * N_USER, (lt + 1) * N_USER)
                        eng = nc.vector if lt < 5 else nc.gpsimd
                        eng.tensor_scalar(out=PT[:, psl], in0=Tb16[:],
                                          scalar1=g32[:, lt:lt + 1],
                                          scalar2=None, op0=OP.max)
                        nc.vector.tensor_tensor(out=PT[:, psl], in0=PT[:, psl],
                                                in1=Mk(lt), op=OP.mult)
                    for i in range(4):
                        ut = half * 4 + i
                        usl = slice(lt * N_USER + ut * P,
                                    lt * N_USER + (ut + 1) * P)
                        nc.tensor.matmul(tiles[i][:], PT[:, usl], whext[lt][:],
                                         start=(lt == 0), stop=False)
                for i in range(4):
                    ut = half * 4 + i
                    nc.tensor.matmul(tiles[i][:],
                                     nothas[0:1, ut * P:(ut + 1) * P],
                                     mw0e[:], start=False, stop=True)
                    nc.vector.reciprocal(rcol[ut][:], tiles[i][:, HD:HD + 1])
                    nc.scalar.activation(h1u[:, ut * HD:(ut + 1) * HD],
                                         tiles[i][:, 0:HD], AF.Relu,
                                         scale=rcol[ut][:])
                if half == 0:
                    nc.scalar.dma_start(
                        out=d_o01[1].rearrange("(t p) h -> p t h", p=P),
                        in_=h1u[:, 0:4 * HD].rearrange("p (t h) -> p t h", t=4))

            # ---------------- scol ----------------
            for mt in range(2):
                ps = psCol.tile([P, 1], f32, name="pcol", tag="col", bufs=1)
                for ut in range(8):
                    nc.tensor.matmul(
                        ps[:], h1u[:, ut * HD + mt * P:ut * HD + (mt + 1) * P],
                        ones16c[:], start=(ut == 0), stop=(ut == 7))
                nc.scalar.activation(vs1[mt][:, 1:2], ps[:], AF.Copy,
                                     scale=1.0 / 3072.0)
                nc.vector.tensor_tensor(out=vs1[mt][:, 2:3], in0=vs1[mt][:, 0:1],
                                        in1=vs1[mt][:, 1:2], op=OP.add)

            # ---------------- days 2..4 ----------------
            vs_cur = vs1
            for dd in range(3):
                base = 2 + 4 * dd
                vs_n = [work.tile([P, 3], f16, name=f"vs{dd}_{kt}",
                                  tag=f"vs{dd}_{kt}") for kt in range(2)]
                for mt in range(2):
                    ps = psSm.tile([P, 3], f32, name="psm", tag="sm", bufs=1)
                    for kt in range(2):
                        nc.tensor.matmul(ps[:], Wk(kt)[:, mt * P:(mt + 1) * P],
                                         vs_cur[kt][:],
                                         start=(kt == 0), stop=(kt == 1))
                    nc.scalar.activation(rows_out[:, base + mt:base + mt + 1],
                                         ps[:, 0:1], AF.Relu, scale=1.5)
                    nc.scalar.activation(rows_out[:, base + 2 + mt:base + 3 + mt],
                                         ps[:, 2:3], AF.Relu)
                    if dd < 2:
                        nc.scalar.activation(vs_n[mt][:, 0:1], ps[:, 2:3],
                                             AF.Relu, scale=2048.0 / 3072.0)
                        tA = work.tile([P, 1], f32, name=f"tA{dd}_{mt}",
                                       tag=f"tA{dd}_{mt}")
                        tB = work.tile([P, 1], f32, name=f"tB{dd}_{mt}",
                                       tag=f"tB{dd}_{mt}")
                        nc.scalar.activation(tA[:], ps[:, 0:1], AF.Relu,
                                             scale=necol[:, 2 * dd:2 * dd + 1])
                        nc.scalar.activation(tB[:], ps[:, 2:3], AF.Relu,
                                             scale=necol[:, 2 * dd + 1:2 * dd + 2])
                        nc.vector.tensor_tensor(out=vs_n[mt][:, 1:2], in0=tA[:],
                                                in1=tB[:], op=OP.add)
                        nc.vector.tensor_tensor(out=vs_n[mt][:, 2:3],
                                                in0=vs_n[mt][:, 0:1],
                                                in1=vs_n[mt][:, 1:2], op=OP.add)
                vs_cur = vs_n
            nc.scalar.dma_start(out=d_rows[:, :], in_=rows_out[:])

    return nc


def _host_prep(x_loc, mob_links, text_links, W, a):
    """Index-derived preprocessing -> per-core input maps."""
    import ml_dtypes
    f8 = ml_dtypes.float8_e4m3

    x_loc = np.ascontiguousarray(x_loc, np.float32)
    W = np.ascontiguousarray(W, np.float32)
    a = np.ascontiguousarray(a, np.float32)
    mob = np.asarray(mob_links)
    text = np.asarray(text_links)

    shared = {
        "W16": W.astype(np.float16),
        "WT16": np.ascontiguousarray(W.T).astype(np.float16),
        "acol2": np.concatenate([a[:HD], a[HD:]], axis=1).astype(np.float16),
        "xlocT16": np.ascontiguousarray(x_loc.T).astype(np.float16),
        "xloc16": x_loc.astype(np.float16),
    }

    in_maps = []
    for c in range(NCORES):
        b, r = c // 2, c % 2
        rot = r * 512
        u0 = np.concatenate([mob[b, 0, :, 0], text[b, 0, :, 0]]).astype(np.int64)
        l0 = np.concatenate([mob[b, 0, :, 1], text[b, 0, :, 1]]).astype(np.int64)
        cnt = np.bincount(u0, minlength=N_USER).astype(np.float32)
        A = np.zeros((N_USER, N_LOC), np.float32)
        np.add.at(A, (u0, l0), 1.0)
        Ahat = A / np.maximum(cnt, 1.0)[:, None]
        Mb = np.zeros((N_USER, N_LOC), np.float32)
        Tb = np.zeros((N_USER, N_LOC), np.float32)
        Mb[mob[b, 0, :, 0], mob[b, 0, :, 1]] = 1.0
        Tb[text[b, 0, :, 0], text[b, 0, :, 1]] = 1.0
        M = Mb + Tb
        has0 = (cnt > 0).astype(np.float32)
        n_with = max(float(has0.sum()), 1.0)
        nh_cnt = float(N_USER) - float(has0.sum())
        hw = has0 / n_with

        w_mwe = (hw[:, None] * Ahat).sum(0)
        w_sxu = Ahat.sum(0) + nh_cnt * w_mwe
        w_h0 = (w_sxu + 2.0) / 3072.0

        ne = np.zeros(3, np.float32)
        for dd in range(3):
            us = np.concatenate([mob[b, dd + 1, :, 0], text[b, dd + 1, :, 0]])
            hasE = np.zeros(N_USER, np.float32)
            hasE[us] = 1.0
            ne[dd] = hasE.sum()
        nec = np.zeros(6, np.float32)
        for dd in range(3):
            nec[2 * dd] = ne[dd] / 2048.0
            nec[2 * dd + 1] = (N_USER - ne[dd]) / 3072.0

        def rollu(x, axis=0):
            return np.roll(x, -rot, axis=axis)

        m = dict(shared)
        m.update({
            "Ahat8": np.ascontiguousarray(rollu(Ahat, 0).T).astype(f8),
            "MT16": np.ascontiguousarray(rollu(M, 0).T).astype(np.float16),
            "nothas16": rollu(1.0 - has0)[None, :].astype(np.float16),
            "wcols": np.stack([w_h0, w_mwe], axis=1).astype(np.float16),
            "necol": np.tile(nec[None, :], (P, 1)).astype(np.float32),
        })
        in_maps.append(m)
    return in_maps


def kernel(**inputs):
    from concourse.bass_utils import run_bass_kernel_spmd

    if "nc" not in _CACHE:
        _CACHE["nc"] = _build_nc()
    nc = _CACHE["nc"]

    x_loc = np.ascontiguousarray(inputs["x_loc"], np.float32)
    mob = np.asarray(inputs["mob_links"])
    text = np.asarray(inputs["text_links"])

    in_maps = _host_prep(inputs["x_loc"], inputs["mob_links"],
                         inputs["text_links"], inputs["W"], inputs["a"])
    res = run_bass_kernel_spmd(nc, in_maps, core_ids=list(range(NCORES)))

    out = np.zeros((B, D, N_USER + 2 * N_LOC, HD), np.float32)
    for c in range(NCORES):
        b, r = c // 2, c % 2
        o01 = np.asarray(res.results[c]["o01"], np.float32)
        out[b, 0, r * 512:(r + 1) * 512] = o01[0]
        out[b, 1, r * 512:(r + 1) * 512] = o01[1]
        if r == 0:
            rows = np.asarray(res.results[c]["rows"], np.float32)
            out[b, 0, N_USER:N_USER + N_LOC] = x_loc
            out[b, 0, N_USER + N_LOC:] = x_loc
            v1 = np.concatenate([rows[:, 0], rows[:, 1]])
            out[b, 1, N_USER:] = v1[None, :]
            for dd, d in enumerate((2, 3, 4)):
                s = 2 + 4 * dd
                r1 = np.concatenate([rows[:, s], rows[:, s + 1]])
                vn = np.concatenate([rows[:, s + 2], rows[:, s + 3]])
                us = np.concatenate([mob[b, d - 1, :, 0], text[b, d - 1, :, 0]])
                hasE = np.zeros(N_USER, bool)
                hasE[us] = True
                out[b, d, :N_USER] = np.where(hasE[:, None], r1[None, :],
                                              vn[None, :])
                out[b, d, N_USER:] = vn[None, :]
    return out


# revision 13
# speedup vs baseline: 2.1634x; 1.8452x over previous
"""Trainium2 Bass kernel for nn_BiGNN_53772990546511.

Math (validated vs reference in mathcheck.py, global l2 rel ~2.5e-4):
  - relu(elu(x)) == relu(x).
  - Day-1 attention collapses to users x locs with multiplicity mask M.
    Softmax is invariant to per-column scaling, so
      exp(leaky(f1_u + f2_l)) ~ max(g_l, t_u) scaled per-row by c_l,
    with g = exp(0.8 f2 + B), t = exp(-0.8 f1 + B), c = exp(0.2 f2 + G).
    The c_l row factor is folded into the whext rows (including the ones
    column), so PT = max(g_l, t_u) * M needs only 2 elementwise ops/tile.
  - Days 2..4 are rank-2 row algebra kept in column form on device (no
    transposes); only 10 row-vectors leave the device and the host
    broadcasts them into the full output (pure gather/unshard).
  - All global sums over users (mwe, mean h0) are rank-1: host prepares
    index-derived weight vectors (w_mwe, w_h0) so phase-2 only computes
    the 512 own-user rows.
  - Device writes per core: 0.5MB (day0+day1 own user halves, f16) +
    rows; everything else is broadcast host-side from row vectors.

Sharding: 8 cores = 4 batches x 2 user-halves.  Odd cores get index
tensors rotated by 512 along users so the program is SPMD-uniform.
"""
import numpy as np

N_USER = 1024
N_LOC = 1024
DM = 256
HD = 256
B = 4
D = 5
E = 4096
ALPHA = 0.2
BETA = -1.0
GAMMA = -0.5
P = 128
NCORES = 8

_CACHE = {}


# --------------------------------------------------------------------------
# Workarounds for this walrus build's 1-sync-wait-per-instruction limit.
# --------------------------------------------------------------------------
def _apply_tile_patch():
    import concourse.tile as tile
    from concourse.tile_sem_assignment import tick_to_sem

    if not getattr(tile.TileContext, "_drain_patched", False):
        def _patched(self, tick_clock, wait_clock):
            nc = self.nc
            gc = tick_clock.global_clock
            for proc, sem in self.sems.allocated().items():
                t = gc[proc]
                if t and t > 0:
                    nc.sync.nop().wait_op(sem, tick_to_sem(t, proc), "sem-ge")
            nc.sync.drain()
            nc.all_engine_barrier()
            popped = nc._tile_sem_poison_stack.pop()
            assert popped is self._sem_poison
            nc.clear_and_free_semaphores(list(self.sems.allocated().values()))
            nc.all_engine_barrier()

        tile.TileContext._drain_and_barrier = _patched
        tile.TileContext._drain_patched = True

    import json as _json
    import concourse.bass_utils as _bu
    import concourse.bass2jax as _b2j

    if not getattr(_bu, "_wait_split_patched", False):
        _orig_compile = _bu.compile_bir_kernel

        def _split_waits(bir_json):
            j = _json.loads(bir_json)
            nid = [0]
            for fn in j.get("functions", []):
                for bb in fn.get("blocks", []):
                    out = []
                    for inst in bb.get("instructions", []):
                        si = inst.get("sync_info") or {}
                        ow = si.get("on_wait") or []
                        if len(ow) > 1:
                            for w in ow[:-1]:
                                nid[0] += 1
                                out.append({
                                    "debug": inst.get("debug", 0),
                                    "engine": inst.get("engine", "SP"),
                                    "ins": [],
                                    "name": f"WSPL-{nid[0]}",
                                    "opcode": "NoOp",
                                    "outs": [],
                                    "sync_info": {"on_update": [],
                                                  "on_wait": [w]},
                                })
                            si["on_wait"] = [ow[-1]]
                        out.append(inst)
                    bb["instructions"] = out
            return _json.dumps(j).encode()

        def _patched_compile(bir_json, tmpdir, neff_name="file.neff"):
            return _orig_compile(_split_waits(bir_json), tmpdir,
                                 neff_name=neff_name)

        _bu.compile_bir_kernel = _patched_compile
        _b2j.compile_bir_kernel = _patched_compile
        _bu._wait_split_patched = True


def _build_nc():
    import contextlib
    import concourse.bass as bass
    import concourse.tile as tile
    from concourse import mybir

    _apply_tile_patch()
    f32 = mybir.dt.float32
    f16 = mybir.dt.float16
    f8 = mybir.dt.float8e4
    AF = mybir.ActivationFunctionType
    OP = mybir.AluOpType

    nc = bass.Bass()

    # ---------------- DRAM tensors ----------------
    d_W16 = nc.dram_tensor("W16", [DM, HD], f16, kind="ExternalInput")
    d_WT16 = nc.dram_tensor("WT16", [HD, DM], f16, kind="ExternalInput")
    d_acol2 = nc.dram_tensor("acol2", [HD, 2], f16, kind="ExternalInput")
    d_wcols = nc.dram_tensor("wcols", [N_LOC, 2], f16, kind="ExternalInput")
    d_nothas = nc.dram_tensor("nothas16", [1, N_USER], f16, kind="ExternalInput")
    d_necol = nc.dram_tensor("necol", [P, 6], f32, kind="ExternalInput")
    d_xlocT = nc.dram_tensor("xlocT16", [DM, N_LOC], f16, kind="ExternalInput")
    d_xloc = nc.dram_tensor("xloc16", [N_LOC, DM], f16, kind="ExternalInput")
    d_A8 = nc.dram_tensor("Ahat8", [N_LOC, N_USER], f8, kind="ExternalInput")
    d_MT = nc.dram_tensor("MT16", [N_LOC, N_USER], f16, kind="ExternalInput")
    d_o01 = nc.dram_tensor("o01", [2, 512, HD], f16, kind="ExternalOutput")
    d_rows = nc.dram_tensor("rows", [P, 14], f32, kind="ExternalOutput")

    with tile.TileContext(nc) as tc:
        with contextlib.ExitStack() as ctx:
            persist = ctx.enter_context(tc.tile_pool(name="persist", bufs=1))
            work = ctx.enter_context(tc.tile_pool(name="work", bufs=1))
            psBig = ctx.enter_context(tc.tile_pool(name="psBig", bufs=2, space="PSUM"))
            ps6 = ctx.enter_context(tc.tile_pool(name="ps6", bufs=4, space="PSUM"))
            psSm = psBig
            psCol = psBig

            # ------------- input loads, in arrival-priority order -------------
            def big_load(dst, dram, t):
                src = dram.rearrange("(t p) u -> p t u", p=P)
                nc.sync.dma_start(
                    out=dst[:].rearrange("p (t u) -> p t u", t=t), in_=src)

            Wsb = persist.tile([P, 2 * HD], f16, name="Wsb")
            big_load(Wsb, d_W16[:], 2)
            WTsb = persist.tile([P, 2 * DM], f16, name="WTsb")
            big_load(WTsb, d_WT16[:], 2)
            acol = persist.tile([P, 4], f16, name="acol")
            big_load(acol, d_acol2[:], 2)
            wcol = persist.tile([P, 16], f16, name="wcol")
            big_load(wcol, d_wcols[:], 8)
            nothas = persist.tile([1, N_USER], f16, name="nothas")
            nc.sync.dma_start(out=nothas[:], in_=d_nothas[:])
            necol = persist.tile([P, 6], f32, name="necol")
            nc.sync.dma_start(out=necol[:], in_=d_necol[:])
            xlocT = persist.tile([P, 2 * N_LOC], f16, name="xlocT")
            big_load(xlocT, d_xlocT[:], 2)
            A8 = persist.tile([P, 8 * N_USER], f8, name="A8")
            big_load(A8, d_A8[:], 8)
            xloc = persist.tile([P, 8 * DM], f16, name="xloc")
            big_load(xloc, d_xloc[:], 8)
            MT = persist.tile([P, 8 * N_USER], f16, name="MT")
            big_load(MT, d_MT[:], 8)

            def Wk(kt):
                return Wsb[:, kt * HD:(kt + 1) * HD]

            def WTk(kt):
                return WTsb[:, kt * DM:(kt + 1) * DM]

            def xT(kt, sl):
                return xlocT[:, kt * N_LOC:(kt + 1) * N_LOC][:, sl]

            def xl(lt):
                return xloc[:, lt * DM:(lt + 1) * DM]

            def Ah(lt, sl):
                return A8[:, lt * N_USER:(lt + 1) * N_USER][:, sl]

            def Mk(lt):
                return MT[:, lt * N_USER:(lt + 1) * N_USER]

            # constants
            ones16r = persist.tile([1, P], f16, name="ones16r")
            nc.vector.memset(ones16r[:], 1.0)
            one11 = persist.tile([1, 1], f16, name="one11")
            nc.vector.memset(one11[:], 1.0)
            ones16c = persist.tile([P, 1], f16, name="ones16c")
            nc.vector.memset(ones16c[:], 1.0)
            cBETA = persist.tile([P, 1], f32, name="cBETA")
            nc.vector.memset(cBETA[:], BETA)
            cGAMMA = persist.tile([P, 1], f32, name="cGAMMA")
            nc.vector.memset(cGAMMA[:], GAMMA)

            # persistent intermediates
            wa12 = [persist.tile([P, 2], f16, name=f"wa12_{kt}") for kt in range(2)]
            xw1c = persist.tile([P, 8], f16, name="xw1c")
            xw2c = persist.tile([P, 8], f32, name="xw2c")
            g32 = persist.tile([P, 8], f32, name="g32")
            c32 = persist.tile([P, 8], f32, name="c32")
            whext = [persist.tile([P, HD + 1], f16, name=f"whext{lt}")
                     for lt in range(8)]
            mh0row = persist.tile([1, DM], f16, name="mh0row")
            mwe16 = persist.tile([1, DM], f16, name="mwe16")
            mh0c = persist.tile([P, 2], f16, name="mh0c")
            mw0e = persist.tile([1, HD + 1], f16, name="mw0e")
            t16 = persist.tile([1, N_USER], f16, name="t16")
            Tb16 = persist.tile([P, N_USER], f16, name="Tb16")
            PT = persist.tile([P, 8 * N_USER], f16, name="PT")
            day0 = persist.tile([P, 4 * HD], f16, name="day0")
            h1u = persist.tile([P, 8 * HD], f16, name="h1u")
            rows_out = persist.tile([P, 14], f32, name="rows_out")
            rcol = [persist.tile([P, 1], f32, name=f"rcol{ut}") for ut in range(8)]

            # ---------------- wa12 = [W a1 | W a2] as cols ----------------
            for kt in range(2):
                ps = psSm.tile([P, 2], f32, name="psm", tag="sm", bufs=2)
                for ht in range(2):
                    nc.tensor.matmul(ps[:], WTk(ht)[:, kt * P:(kt + 1) * P],
                                     acol[:, ht * 2:ht * 2 + 2],
                                     start=(ht == 0), stop=(ht == 1))
                nc.scalar.activation(wa12[kt][:], ps[:], AF.Copy)

            # ---------------- xw12 cols per loc tile ----------------
            for lt in range(8):
                ps = psSm.tile([P, 2], f32, name="psm", tag="sm", bufs=2)
                for kt in range(2):
                    nc.tensor.matmul(ps[:], xT(kt, slice(lt * P, (lt + 1) * P)),
                                     wa12[kt][:], start=(kt == 0), stop=(kt == 1))
                nc.vector.tensor_copy(xw1c[:, lt:lt + 1], ps[:, 0:1])
                nc.vector.tensor_copy(xw2c[:, lt:lt + 1], ps[:, 1:2])
            nc.scalar.activation(g32[:], xw2c[:], AF.Exp, scale=0.8, bias=cBETA[:])
            nc.scalar.activation(c32[:], xw2c[:], AF.Exp, scale=0.2, bias=cGAMMA[:])

            # ---------------- whext_c = c_l * [Wh_l | 1] ----------------
            for lt in range(8):
                ps = psBig.tile([P, HD], f32, name="pwh", tag="big")
                for kt in range(2):
                    nc.tensor.matmul(ps[:], xT(kt, slice(lt * P, (lt + 1) * P)),
                                     Wk(kt), start=(kt == 0), stop=(kt == 1))
                nc.scalar.activation(whext[lt][:, 0:HD], ps[:], AF.Copy,
                                     scale=c32[:, lt:lt + 1])
                nc.vector.tensor_copy(whext[lt][:, HD:HD + 1], c32[:, lt:lt + 1])

            # ---------------- mh0 / mwe rows ----------------
            pmr = psBig.tile([1, DM], f32, name="pmr", tag="big")
            for lt in range(8):
                nc.tensor.matmul(pmr[:], wcol[:, lt * 2:lt * 2 + 1], xl(lt),
                                 start=(lt == 0), stop=(lt == 7))
            nc.scalar.activation(mh0row[:], pmr[:], AF.Copy)
            pmr2 = psBig.tile([1, DM], f32, name="pmr2", tag="big")
            for lt in range(8):
                nc.tensor.matmul(pmr2[:], wcol[:, lt * 2 + 1:lt * 2 + 2], xl(lt),
                                 start=(lt == 0), stop=(lt == 7))
            nc.scalar.activation(mwe16[:], pmr2[:], AF.Copy)
            for mt in range(2):
                ps = psSm.tile([P, 1], f32, name="psm", tag="sm", bufs=2)
                nc.tensor.matmul(ps[:], mh0row[0:1, mt * P:(mt + 1) * P],
                                 one11[:], start=True, stop=True)
                nc.vector.tensor_copy(mh0c[:, mt:mt + 1], ps[:])
            pmw = psBig.tile([1, HD], f32, name="pmw", tag="big")
            for kt in range(2):
                nc.tensor.matmul(pmw[:], mh0c[:, kt:kt + 1], Wk(kt),
                                 start=(kt == 0), stop=(kt == 1))
            nc.scalar.activation(mw0e[:, 0:HD], pmw[:], AF.Copy)
            nc.vector.memset(mw0e[:, HD:HD + 1], 1.0)

            # mw0 cols -> v1 (day-1 loc row) and vs-chain init
            vs1 = [work.tile([P, 3], f16, name=f"vs1_{kt}", tag=f"vs1_{kt}")
                   for kt in range(2)]
            for mt in range(2):
                ps = psCol.tile([P, 1], f32, name="pcol", tag="sm", bufs=2)
                for kt in range(2):
                    nc.tensor.matmul(ps[:], Wk(kt)[:, mt * P:(mt + 1) * P],
                                     mh0c[:, kt:kt + 1],
                                     start=(kt == 0), stop=(kt == 1))
                nc.scalar.activation(rows_out[:, mt:mt + 1], ps[:], AF.Relu)
                nc.scalar.activation(vs1[mt][:, 0:1], ps[:], AF.Relu,
                                     scale=2048.0 / 3072.0)

            # ---------------- f1row -> t -> Tb ----------------
            for ch in range(2):
                csl = slice(ch * 512, (ch + 1) * 512)
                pf = psBig.tile([1, 512], f32, name="pf1", tag="big")
                for lt in range(8):
                    nc.tensor.matmul(pf[:], xw1c[:, lt:lt + 1], Ah(lt, csl),
                                     start=(lt == 0), stop=(lt == 7))
                nc.scalar.activation(t16[0:1, csl], pf[:], AF.Exp,
                                     scale=-0.8, bias=cBETA[0:1, :])
                ptb = psBig.tile([P, 512], f32, name="ptb", tag="big")
                nc.tensor.matmul(ptb[:], ones16r[:], t16[0:1, csl],
                                 start=True, stop=True)
                nc.scalar.activation(Tb16[:, csl], ptb[:], AF.Copy)

            # ---------------- phase 2: x_user own half ----------------
            for ut in range(4):
                ps = psBig.tile([P, DM], f32, name="px", tag="big")
                for lt in range(8):
                    nc.tensor.matmul(ps[:], Ah(lt, slice(ut * P, (ut + 1) * P)),
                                     xl(lt), start=(lt == 0), stop=False)
                nc.tensor.matmul(ps[:], nothas[0:1, ut * P:(ut + 1) * P],
                                 mwe16[:], start=False, stop=True)
                nc.scalar.activation(day0[:, ut * HD:(ut + 1) * HD], ps[:], AF.Copy)
            nc.scalar.dma_start(
                out=d_o01[0].rearrange("(t p) h -> p t h", p=P),
                in_=day0[:].rearrange("p (t h) -> p t h", t=4))

            # ---------------- PT + phase 6 (interleaved over loc tiles) -----
            for half in range(2):
                tiles = [ps6.tile([P, HD + 1], f32, name=f"p6_{half}_{i}",
                                  tag="p6") for i in range(4)]
                for lt in range(8):
                    if half == 0:
                        psl = slice(lt * N_USER, (lt + 1) * N_USER)
                        nc.vector.tensor_scalar(out=PT[:, psl], in0=Tb16[:],
                                          scalar1=g32[:, lt:lt + 1],
                                          scalar2=None, op0=OP.max)
                        nc.vector.tensor_tensor(out=PT[:, psl], in0=PT[:, psl],
                                                in1=Mk(lt), op=OP.mult)
                    for i in range(4):
                        ut = half * 4 + i
                        usl = slice(lt * N_USER + ut * P,
                                    lt * N_USER + (ut + 1) * P)
                        nc.tensor.matmul(tiles[i][:], PT[:, usl], whext[lt][:],
                                         start=(lt == 0), stop=False)
                for i in range(4):
                    ut = half * 4 + i
                    nc.tensor.matmul(tiles[i][:],
                                     nothas[0:1, ut * P:(ut + 1) * P],
                                     mw0e[:], start=False, stop=True)
                    nc.vector.reciprocal(rcol[ut][:], tiles[i][:, HD:HD + 1])
                    nc.scalar.activation(h1u[:, ut * HD:(ut + 1) * HD],
                                         tiles[i][:, 0:HD], AF.Relu,
                                         scale=rcol[ut][:])
                if half == 0:
                    nc.scalar.dma_start(
                        out=d_o01[1].rearrange("(t p) h -> p t h", p=P),
                        in_=h1u[:, 0:4 * HD].rearrange("p (t h) -> p t h", t=4))

            # ---------------- scol ----------------
            for mt in range(2):
                ps = psCol.tile([P, 1], f32, name="pcol", tag="sm", bufs=2)
                for ut in range(8):
                    nc.tensor.matmul(
                        ps[:], h1u[:, ut * HD + mt * P:ut * HD + (mt + 1) * P],
                        ones16c[:], start=(ut == 0), stop=(ut == 7))
                nc.scalar.activation(vs1[mt][:, 1:2], ps[:], AF.Copy,
                                     scale=1.0 / 3072.0)
                nc.vector.tensor_tensor(out=vs1[mt][:, 2:3], in0=vs1[mt][:, 0:1],
                                        in1=vs1[mt][:, 1:2], op=OP.add)

            # ---------------- days 2..4 ----------------
            vs_cur = vs1
            for dd in range(3):
                base = 2 + 4 * dd
                vs_n = [work.tile([P, 3], f16, name=f"vs{dd}_{kt}",
                                  tag=f"vs{dd}_{kt}") for kt in range(2)]
                for mt in range(2):
                    ps = psSm.tile([P, 3], f32, name="psm", tag="sm", bufs=2)
                    for kt in range(2):
                        nc.tensor.matmul(ps[:], Wk(kt)[:, mt * P:(mt + 1) * P],
                                         vs_cur[kt][:],
                                         start=(kt == 0), stop=(kt == 1))
                    nc.scalar.activation(rows_out[:, base + mt:base + mt + 1],
                                         ps[:, 0:1], AF.Relu, scale=1.5)
                    nc.scalar.activation(rows_out[:, base + 2 + mt:base + 3 + mt],
                                         ps[:, 2:3], AF.Relu)
                    if dd < 2:
                        nc.scalar.activation(vs_n[mt][:, 0:1], ps[:, 2:3],
                                             AF.Relu, scale=2048.0 / 3072.0)
                        tA = work.tile([P, 1], f32, name=f"tA{dd}_{mt}",
                                       tag=f"tA{dd}_{mt}")
                        tB = work.tile([P, 1], f32, name=f"tB{dd}_{mt}",
                                       tag=f"tB{dd}_{mt}")
                        nc.scalar.activation(tA[:], ps[:, 0:1], AF.Relu,
                                             scale=necol[:, 2 * dd:2 * dd + 1])
                        nc.scalar.activation(tB[:], ps[:, 2:3], AF.Relu,
                                             scale=necol[:, 2 * dd + 1:2 * dd + 2])
                        nc.vector.tensor_tensor(out=vs_n[mt][:, 1:2], in0=tA[:],
                                                in1=tB[:], op=OP.add)
                        nc.vector.tensor_tensor(out=vs_n[mt][:, 2:3],
                                                in0=vs_n[mt][:, 0:1],
                                                in1=vs_n[mt][:, 1:2], op=OP.add)
                vs_cur = vs_n
            nc.scalar.dma_start(out=d_rows[:, :], in_=rows_out[:])

    return nc


def _host_prep(x_loc, mob_links, text_links, W, a):
    """Index-derived preprocessing -> per-core input maps."""
    import ml_dtypes
    f8 = ml_dtypes.float8_e4m3

    x_loc = np.ascontiguousarray(x_loc, np.float32)
    W = np.ascontiguousarray(W, np.float32)
    a = np.ascontiguousarray(a, np.float32)
    mob = np.asarray(mob_links)
    text = np.asarray(text_links)

    shared = {
        "W16": W.astype(np.float16),
        "WT16": np.ascontiguousarray(W.T).astype(np.float16),
        "acol2": np.concatenate([a[:HD], a[HD:]], axis=1).astype(np.float16),
        "xlocT16": np.ascontiguousarray(x_loc.T).astype(np.float16),
        "xloc16": x_loc.astype(np.float16),
    }

    in_maps = []
    for c in range(NCORES):
        b, r = c // 2, c % 2
        rot = r * 512
        u0 = np.concatenate([mob[b, 0, :, 0], text[b, 0, :, 0]]).astype(np.int64)
        l0 = np.concatenate([mob[b, 0, :, 1], text[b, 0, :, 1]]).astype(np.int64)
        cnt = np.bincount(u0, minlength=N_USER).astype(np.float32)
        A = np.zeros((N_USER, N_LOC), np.float32)
        np.add.at(A, (u0, l0), 1.0)
        Ahat = A / np.maximum(cnt, 1.0)[:, None]
        Mb = np.zeros((N_USER, N_LOC), np.float32)
        Tb = np.zeros((N_USER, N_LOC), np.float32)
        Mb[mob[b, 0, :, 0], mob[b, 0, :, 1]] = 1.0
        Tb[text[b, 0, :, 0], text[b, 0, :, 1]] = 1.0
        M = Mb + Tb
        has0 = (cnt > 0).astype(np.float32)
        n_with = max(float(has0.sum()), 1.0)
        nh_cnt = float(N_USER) - float(has0.sum())
        hw = has0 / n_with

        w_mwe = (hw[:, None] * Ahat).sum(0)
        w_sxu = Ahat.sum(0) + nh_cnt * w_mwe
        w_h0 = (w_sxu + 2.0) / 3072.0

        ne = np.zeros(3, np.float32)
        for dd in range(3):
            us = np.concatenate([mob[b, dd + 1, :, 0], text[b, dd + 1, :, 0]])
            hasE = np.zeros(N_USER, np.float32)
            hasE[us] = 1.0
            ne[dd] = hasE.sum()
        nec = np.zeros(6, np.float32)
        for dd in range(3):
            nec[2 * dd] = ne[dd] / 2048.0
            nec[2 * dd + 1] = (N_USER - ne[dd]) / 3072.0

        def rollu(x, axis=0):
            return np.roll(x, -rot, axis=axis)

        m = dict(shared)
        m.update({
            "Ahat8": np.ascontiguousarray(rollu(Ahat, 0).T).astype(f8),
            "MT16": np.ascontiguousarray(rollu(M, 0).T).astype(np.float16),
            "nothas16": rollu(1.0 - has0)[None, :].astype(np.float16),
            "wcols": np.stack([w_h0, w_mwe], axis=1).astype(np.float16),
            "necol": np.tile(nec[None, :], (P, 1)).astype(np.float32),
        })
        in_maps.append(m)
    return in_maps


def kernel(**inputs):
    from concourse.bass_utils import run_bass_kernel_spmd

    if "nc" not in _CACHE:
        _CACHE["nc"] = _build_nc()
    nc = _CACHE["nc"]

    x_loc = np.ascontiguousarray(inputs["x_loc"], np.float32)
    mob = np.asarray(inputs["mob_links"])
    text = np.asarray(inputs["text_links"])

    in_maps = _host_prep(inputs["x_loc"], inputs["mob_links"],
                         inputs["text_links"], inputs["W"], inputs["a"])
    res = run_bass_kernel_spmd(nc, in_maps, core_ids=list(range(NCORES)))

    out = np.zeros((B, D, N_USER + 2 * N_LOC, HD), np.float32)
    for c in range(NCORES):
        b, r = c // 2, c % 2
        o01 = np.asarray(res.results[c]["o01"], np.float32)
        out[b, 0, r * 512:(r + 1) * 512] = o01[0]
        out[b, 1, r * 512:(r + 1) * 512] = o01[1]
        if r == 0:
            rows = np.asarray(res.results[c]["rows"], np.float32)
            out[b, 0, N_USER:N_USER + N_LOC] = x_loc
            out[b, 0, N_USER + N_LOC:] = x_loc
            v1 = np.concatenate([rows[:, 0], rows[:, 1]])
            out[b, 1, N_USER:] = v1[None, :]
            for dd, d in enumerate((2, 3, 4)):
                s = 2 + 4 * dd
                r1 = np.concatenate([rows[:, s], rows[:, s + 1]])
                vn = np.concatenate([rows[:, s + 2], rows[:, s + 3]])
                us = np.concatenate([mob[b, d - 1, :, 0], text[b, d - 1, :, 0]])
                hasE = np.zeros(N_USER, bool)
                hasE[us] = True
                out[b, d, :N_USER] = np.where(hasE[:, None], r1[None, :],
                                              vn[None, :])
                out[b, d, N_USER:] = vn[None, :]
    return out


# revision 15
# speedup vs baseline: 2.1928x; 1.0136x over previous
"""Trainium2 Bass kernel for nn_BiGNN_53772990546511.

Math (validated vs reference in mathcheck.py, global l2 rel ~2.5e-4):
  - relu(elu(x)) == relu(x).
  - Day-1 attention collapses to users x locs with multiplicity mask M.
    Softmax is invariant to per-column scaling, so
      exp(leaky(f1_u + f2_l)) ~ max(g_l, t_u) scaled per-row by c_l,
    with g = exp(0.8 f2 + B), t = exp(-0.8 f1 + B), c = exp(0.2 f2 + G).
    The c_l row factor is folded into the whext rows (including the ones
    column), so PT = max(g_l, t_u) * M needs only 2 elementwise ops/tile.
  - Days 2..4 are rank-2 row algebra kept in column form on device (no
    transposes); only 10 row-vectors leave the device and the host
    broadcasts them into the full output (pure gather/unshard).
  - All global sums over users (mwe, mean h0) are rank-1: host prepares
    index-derived weight vectors (w_mwe, w_h0) so phase-2 only computes
    the 512 own-user rows.
  - Device writes per core: 0.5MB (day0+day1 own user halves, f16) +
    rows; everything else is broadcast host-side from row vectors.

Sharding: 8 cores = 4 batches x 2 user-halves.  Odd cores get index
tensors rotated by 512 along users so the program is SPMD-uniform.
"""
import numpy as np

N_USER = 1024
N_LOC = 1024
DM = 256
HD = 256
B = 4
D = 5
E = 4096
ALPHA = 0.2
BETA = -1.0
GAMMA = -0.5
P = 128
NCORES = 8

_CACHE = {}


# --------------------------------------------------------------------------
# Workarounds for this walrus build's 1-sync-wait-per-instruction limit.
# --------------------------------------------------------------------------
def _apply_tile_patch():
    import concourse.tile as tile
    from concourse.tile_sem_assignment import tick_to_sem

    if not getattr(tile.TileContext, "_drain_patched", False):
        def _patched(self, tick_clock, wait_clock):
            nc = self.nc
            gc = tick_clock.global_clock
            for proc, sem in self.sems.allocated().items():
                t = gc[proc]
                if t and t > 0:
                    nc.sync.nop().wait_op(sem, tick_to_sem(t, proc), "sem-ge")
            nc.sync.drain()
            nc.all_engine_barrier()
            popped = nc._tile_sem_poison_stack.pop()
            assert popped is self._sem_poison
            nc.clear_and_free_semaphores(list(self.sems.allocated().values()))
            nc.all_engine_barrier()

        tile.TileContext._drain_and_barrier = _patched
        tile.TileContext._drain_patched = True

    import json as _json
    import concourse.bass_utils as _bu
    import concourse.bass2jax as _b2j

    if not getattr(_bu, "_wait_split_patched", False):
        _orig_compile = _bu.compile_bir_kernel

        def _split_waits(bir_json):
            j = _json.loads(bir_json)
            nid = [0]
            for fn in j.get("functions", []):
                for bb in fn.get("blocks", []):
                    out = []
                    for inst in bb.get("instructions", []):
                        si = inst.get("sync_info") or {}
                        ow = si.get("on_wait") or []
                        if len(ow) > 1:
                            for w in ow[:-1]:
                                nid[0] += 1
                                out.append({
                                    "debug": inst.get("debug", 0),
                                    "engine": inst.get("engine", "SP"),
                                    "ins": [],
                                    "name": f"WSPL-{nid[0]}",
                                    "opcode": "NoOp",
                                    "outs": [],
                                    "sync_info": {"on_update": [],
                                                  "on_wait": [w]},
                                })
                            si["on_wait"] = [ow[-1]]
                        out.append(inst)
                    bb["instructions"] = out
            return _json.dumps(j).encode()

        def _patched_compile(bir_json, tmpdir, neff_name="file.neff"):
            return _orig_compile(_split_waits(bir_json), tmpdir,
                                 neff_name=neff_name)

        _bu.compile_bir_kernel = _patched_compile
        _b2j.compile_bir_kernel = _patched_compile
        _bu._wait_split_patched = True


def _build_nc():
    import contextlib
    import concourse.bass as bass
    import concourse.tile as tile
    from concourse import mybir

    _apply_tile_patch()
    f32 = mybir.dt.float32
    f16 = mybir.dt.float16
    f8 = mybir.dt.float8e4
    AF = mybir.ActivationFunctionType
    OP = mybir.AluOpType

    nc = bass.Bass()

    # ---------------- DRAM tensors (device layout: [128, cols]) ----------------
    d_blob = nc.dram_tensor("blob", [P, 1044], f16, kind="ExternalInput")
    d_nothas = nc.dram_tensor("nothas16", [1, N_USER], f16, kind="ExternalInput")
    d_necol = nc.dram_tensor("necol", [P, 6], f32, kind="ExternalInput")
    d_xlocT = nc.dram_tensor("xlocT16", [P, 2 * N_LOC], f16, kind="ExternalInput")
    d_xloc = nc.dram_tensor("xloc16", [P, 8 * DM], f16, kind="ExternalInput")
    d_A8 = nc.dram_tensor("Ahat8", [P, 8 * N_USER], f8, kind="ExternalInput")
    d_MT = nc.dram_tensor("MT16", [P, 8 * N_USER], f16, kind="ExternalInput")
    d_o0 = nc.dram_tensor("o0", [P, 4 * HD], f16, kind="ExternalOutput")
    d_o1 = nc.dram_tensor("o1", [P, 4 * HD], f16, kind="ExternalOutput")
    d_rows = nc.dram_tensor("rows", [P, 14], f32, kind="ExternalOutput")

    with tile.TileContext(nc) as tc:
        with contextlib.ExitStack() as ctx:
            persist = ctx.enter_context(tc.tile_pool(name="persist", bufs=1))
            work = ctx.enter_context(tc.tile_pool(name="work", bufs=1))
            psBig = ctx.enter_context(tc.tile_pool(name="psBig", bufs=2, space="PSUM"))
            ps6 = ctx.enter_context(tc.tile_pool(name="ps6", bufs=4, space="PSUM"))
            psSm = psBig
            psCol = psBig

            # ------------- input loads, in arrival-priority order -------------
            blob = persist.tile([P, 1044], f16, name="blob")
            nc.sync.dma_start(out=blob[:], in_=d_blob[:])
            Wsb = blob[:, 0:512]
            WTsb = blob[:, 512:1024]
            acol = blob[:, 1024:1028]
            wcol = blob[:, 1028:1044]
            nothas = persist.tile([1, N_USER], f16, name="nothas")
            nc.sync.dma_start(out=nothas[:], in_=d_nothas[:])
            necol = persist.tile([P, 6], f32, name="necol")
            nc.sync.dma_start(out=necol[:], in_=d_necol[:])
            xlocT = persist.tile([P, 2 * N_LOC], f16, name="xlocT")
            nc.sync.dma_start(out=xlocT[:], in_=d_xlocT[:])
            A8 = persist.tile([P, 8 * N_USER], f8, name="A8")
            nc.sync.dma_start(out=A8[:], in_=d_A8[:])
            xloc = persist.tile([P, 8 * DM], f16, name="xloc")
            nc.sync.dma_start(out=xloc[:], in_=d_xloc[:])
            MT = persist.tile([P, 8 * N_USER], f16, name="MT")
            nc.sync.dma_start(out=MT[:, 0:4 * N_USER], in_=d_MT[:, 0:4 * N_USER])
            nc.sync.dma_start(out=MT[:, 4 * N_USER:], in_=d_MT[:, 4 * N_USER:])

            def Wk(kt):
                return Wsb[:, kt * HD:(kt + 1) * HD]

            def WTk(kt):
                return WTsb[:, kt * DM:(kt + 1) * DM]

            def xT(kt, sl):
                s = slice(kt * N_LOC + sl.start, kt * N_LOC + sl.stop)
                return xlocT[:, s]

            def xl(lt):
                return xloc[:, lt * DM:(lt + 1) * DM]

            def Ah(lt, sl):
                s = slice(lt * N_USER + sl.start, lt * N_USER + sl.stop)
                return A8[:, s]

            def Mk(lt):
                return MT[:, lt * N_USER:(lt + 1) * N_USER]

            # constants
            ones16r = persist.tile([1, P], f16, name="ones16r")
            nc.vector.memset(ones16r[:], 1.0)
            one11 = persist.tile([1, 1], f16, name="one11")
            nc.vector.memset(one11[:], 1.0)
            ones16c = persist.tile([P, 1], f16, name="ones16c")
            nc.vector.memset(ones16c[:], 1.0)
            cBETA = persist.tile([P, 1], f32, name="cBETA")
            nc.vector.memset(cBETA[:], BETA)
            cGAMMA = persist.tile([P, 1], f32, name="cGAMMA")
            nc.vector.memset(cGAMMA[:], GAMMA)

            # persistent intermediates
            wa12 = [persist.tile([P, 2], f16, name=f"wa12_{kt}") for kt in range(2)]
            xw1c = persist.tile([P, 8], f16, name="xw1c")
            xw2c = persist.tile([P, 8], f32, name="xw2c")
            g32 = persist.tile([P, 8], f32, name="g32")
            c32 = persist.tile([P, 8], f32, name="c32")
            whext = [persist.tile([P, HD + 1], f16, name=f"whext{lt}")
                     for lt in range(8)]
            mh0row = persist.tile([1, DM], f16, name="mh0row")
            mwe16 = persist.tile([1, DM], f16, name="mwe16")
            mh0c = persist.tile([P, 2], f16, name="mh0c")
            mw0e = persist.tile([1, HD + 1], f16, name="mw0e")
            t16 = persist.tile([1, N_USER], f16, name="t16")
            Tb16 = persist.tile([P, N_USER], f16, name="Tb16")
            PT = persist.tile([P, 8 * N_USER], f16, name="PT")
            day0 = persist.tile([P, 4 * HD], f16, name="day0")
            h1u = persist.tile([P, 8 * HD], f16, name="h1u")
            rows_out = persist.tile([P, 14], f32, name="rows_out")
            rcol = [persist.tile([P, 1], f32, name=f"rcol{ut}") for ut in range(8)]

            # ---------------- wa12 = [W a1 | W a2] as cols ----------------
            for kt in range(2):
                ps = psSm.tile([P, 2], f32, name="psm", tag="sm", bufs=2)
                for ht in range(2):
                    nc.tensor.matmul(ps[:], WTsb[:, ht * DM + kt * P:ht * DM + (kt + 1) * P],
                                     acol[:, ht * 2:ht * 2 + 2],
                                     start=(ht == 0), stop=(ht == 1))
                nc.scalar.activation(wa12[kt][:], ps[:], AF.Copy)

            # ---------------- xw12 cols per loc tile ----------------
            for lt in range(8):
                ps = psSm.tile([P, 2], f32, name="psm", tag="sm", bufs=2)
                for kt in range(2):
                    nc.tensor.matmul(ps[:], xT(kt, slice(lt * P, (lt + 1) * P)),
                                     wa12[kt][:], start=(kt == 0), stop=(kt == 1))
                nc.vector.tensor_copy(xw1c[:, lt:lt + 1], ps[:, 0:1])
                nc.vector.tensor_copy(xw2c[:, lt:lt + 1], ps[:, 1:2])
            nc.scalar.activation(g32[:], xw2c[:], AF.Exp, scale=0.8, bias=cBETA[:])
            nc.scalar.activation(c32[:], xw2c[:], AF.Exp, scale=0.2, bias=cGAMMA[:])

            # ---------------- whext_c = c_l * [Wh_l | 1] ----------------
            for lt in range(8):
                ps = psBig.tile([P, HD], f32, name="pwh", tag="big")
                for kt in range(2):
                    nc.tensor.matmul(ps[:], xT(kt, slice(lt * P, (lt + 1) * P)),
                                     Wk(kt), start=(kt == 0), stop=(kt == 1))
                nc.scalar.activation(whext[lt][:, 0:HD], ps[:], AF.Copy,
                                     scale=c32[:, lt:lt + 1])
                nc.vector.tensor_copy(whext[lt][:, HD:HD + 1], c32[:, lt:lt + 1])

            # ---------------- mh0 / mwe rows ----------------
            pmr = psBig.tile([1, DM], f32, name="pmr", tag="big")
            for lt in range(8):
                nc.tensor.matmul(pmr[:], wcol[:, lt * 2:lt * 2 + 1], xl(lt),
                                 start=(lt == 0), stop=(lt == 7))
            nc.scalar.activation(mh0row[:], pmr[:], AF.Copy)
            pmr2 = psBig.tile([1, DM], f32, name="pmr2", tag="big")
            for lt in range(8):
                nc.tensor.matmul(pmr2[:], wcol[:, lt * 2 + 1:lt * 2 + 2], xl(lt),
                                 start=(lt == 0), stop=(lt == 7))
            nc.scalar.activation(mwe16[:], pmr2[:], AF.Copy)
            for mt in range(2):
                ps = psSm.tile([P, 1], f32, name="psm", tag="sm", bufs=2)
                nc.tensor.matmul(ps[:], mh0row[0:1, mt * P:(mt + 1) * P],
                                 one11[:], start=True, stop=True)
                nc.vector.tensor_copy(mh0c[:, mt:mt + 1], ps[:])
            pmw = psBig.tile([1, HD], f32, name="pmw", tag="big")
            for kt in range(2):
                nc.tensor.matmul(pmw[:], mh0c[:, kt:kt + 1], Wk(kt),
                                 start=(kt == 0), stop=(kt == 1))
            nc.scalar.activation(mw0e[:, 0:HD], pmw[:], AF.Copy)
            nc.vector.memset(mw0e[:, HD:HD + 1], 1.0)

            # mw0 cols -> v1 (day-1 loc row) and vs-chain init
            vs1 = [work.tile([P, 3], f16, name=f"vs1_{kt}", tag=f"vs1_{kt}")
                   for kt in range(2)]
            for mt in range(2):
                ps = psCol.tile([P, 1], f32, name="pcol", tag="sm", bufs=2)
                for kt in range(2):
                    nc.tensor.matmul(ps[:], Wsb[:, kt * HD + mt * P:kt * HD + (mt + 1) * P],
                                     mh0c[:, kt:kt + 1],
                                     start=(kt == 0), stop=(kt == 1))
                nc.scalar.activation(rows_out[:, mt:mt + 1], ps[:], AF.Relu)
                nc.scalar.activation(vs1[mt][:, 0:1], ps[:], AF.Relu,
                                     scale=2048.0 / 3072.0)

            # ---------------- f1row -> t -> Tb ----------------
            for ch in range(2):
                csl = slice(ch * 512, (ch + 1) * 512)
                pf = psBig.tile([1, 512], f32, name="pf1", tag="big")
                for lt in range(8):
                    nc.tensor.matmul(pf[:], xw1c[:, lt:lt + 1], Ah(lt, csl),
                                     start=(lt == 0), stop=(lt == 7))
                nc.scalar.activation(t16[0:1, csl], pf[:], AF.Exp,
                                     scale=-0.8, bias=cBETA[0:1, :])
                ptb = psBig.tile([P, 512], f32, name="ptb", tag="big")
                nc.tensor.matmul(ptb[:], ones16r[:], t16[0:1, csl],
                                 start=True, stop=True)
                nc.scalar.activation(Tb16[:, csl], ptb[:], AF.Copy)

            # ---------------- phase 2: x_user own half ----------------
            for ut in range(4):
                ps = psBig.tile([P, DM], f32, name="px", tag="big")
                for lt in range(8):
                    nc.tensor.matmul(ps[:], Ah(lt, slice(ut * P, (ut + 1) * P)),
                                     xl(lt), start=(lt == 0), stop=False)
                nc.tensor.matmul(ps[:], nothas[0:1, ut * P:(ut + 1) * P],
                                 mwe16[:], start=False, stop=True)
                nc.scalar.activation(day0[:, ut * HD:(ut + 1) * HD], ps[:], AF.Copy)
            nc.scalar.dma_start(out=d_o0[:, :], in_=day0[:])

            # ---------------- PT + phase 6 (interleaved over loc tiles) -----
            for half in range(2):
                tiles = [ps6.tile([P, HD + 1], f32, name=f"p6_{half}_{i}",
                                  tag="p6") for i in range(4)]
                for lt in range(8):
                    if half == 0:
                        psl = slice(lt * N_USER, (lt + 1) * N_USER)
                        nc.vector.scalar_tensor_tensor(
                            out=PT[:, psl], in0=Tb16[:],
                            scalar=g32[:, lt:lt + 1], in1=Mk(lt),
                            op0=OP.max, op1=OP.mult)
                    for i in range(4):
                        ut = half * 4 + i
                        usl = slice(lt * N_USER + ut * P,
                                    lt * N_USER + (ut + 1) * P)
                        nc.tensor.matmul(tiles[i][:], PT[:, usl], whext[lt][:],
                                         start=(lt == 0), stop=False)
                for i in range(4):
                    ut = half * 4 + i
                    nc.tensor.matmul(tiles[i][:],
                                     nothas[0:1, ut * P:(ut + 1) * P],
                                     mw0e[:], start=False, stop=True)
                    nc.vector.reciprocal(rcol[ut][:], tiles[i][:, HD:HD + 1])
                    nc.scalar.activation(h1u[:, ut * HD:(ut + 1) * HD],
                                         tiles[i][:, 0:HD], AF.Relu,
                                         scale=rcol[ut][:])
                if half == 0:
                    nc.scalar.dma_start(out=d_o1[:, :], in_=h1u[:, 0:4 * HD])

            # ---------------- scol ----------------
            for mt in range(2):
                ps = psCol.tile([P, 1], f32, name="pcol", tag="sm", bufs=2)
                for ut in range(8):
                    nc.tensor.matmul(
                        ps[:], h1u[:, ut * HD + mt * P:ut * HD + (mt + 1) * P],
                        ones16c[:], start=(ut == 0), stop=(ut == 7))
                nc.scalar.activation(vs1[mt][:, 1:2], ps[:], AF.Copy,
                                     scale=1.0 / 3072.0)
                nc.vector.tensor_tensor(out=vs1[mt][:, 2:3], in0=vs1[mt][:, 0:1],
                                        in1=vs1[mt][:, 1:2], op=OP.add)

            # ---------------- days 2..4 ----------------
            vs_cur = vs1
            for dd in range(3):
                base = 2 + 4 * dd
                vs_n = [work.tile([P, 3], f16, name=f"vs{dd}_{kt}",
                                  tag=f"vs{dd}_{kt}") for kt in range(2)]
                for mt in range(2):
                    ps = psSm.tile([P, 3], f32, name="psm", tag="sm", bufs=2)
                    for kt in range(2):
                        nc.tensor.matmul(ps[:], Wsb[:, kt * HD + mt * P:kt * HD + (mt + 1) * P],
                                         vs_cur[kt][:],
                                         start=(kt == 0), stop=(kt == 1))
                    nc.scalar.activation(rows_out[:, base + mt:base + mt + 1],
                                         ps[:, 0:1], AF.Relu, scale=1.5)
                    nc.scalar.activation(rows_out[:, base + 2 + mt:base + 3 + mt],
                                         ps[:, 2:3], AF.Relu)
                    if dd < 2:
                        nc.scalar.activation(vs_n[mt][:, 0:1], ps[:, 2:3],
                                             AF.Relu, scale=2048.0 / 3072.0)
                        tA = work.tile([P, 1], f32, name=f"tA{dd}_{mt}",
                                       tag=f"tA{dd}_{mt}")
                        tB = work.tile([P, 1], f32, name=f"tB{dd}_{mt}",
                                       tag=f"tB{dd}_{mt}")
                        nc.scalar.activation(tA[:], ps[:, 0:1], AF.Relu,
                                             scale=necol[:, 2 * dd:2 * dd + 1])
                        nc.scalar.activation(tB[:], ps[:, 2:3], AF.Relu,
                                             scale=necol[:, 2 * dd + 1:2 * dd + 2])
                        nc.vector.tensor_tensor(out=vs_n[mt][:, 1:2], in0=tA[:],
                                                in1=tB[:], op=OP.add)
                        nc.vector.tensor_tensor(out=vs_n[mt][:, 2:3],
                                                in0=vs_n[mt][:, 0:1],
                                                in1=vs_n[mt][:, 1:2], op=OP.add)
                vs_cur = vs_n
            nc.scalar.dma_start(out=d_rows[:, :], in_=rows_out[:])

    return nc


def _host_prep(x_loc, mob_links, text_links, W, a):
    """Index-derived preprocessing -> per-core input maps."""
    import ml_dtypes
    f8 = ml_dtypes.float8_e4m3

    x_loc = np.ascontiguousarray(x_loc, np.float32)
    W = np.ascontiguousarray(W, np.float32)
    a = np.ascontiguousarray(a, np.float32)
    mob = np.asarray(mob_links)
    text = np.asarray(text_links)

    def dev_layout(x):
        t = x.shape[0] // P
        return np.concatenate([x[i * P:(i + 1) * P] for i in range(t)], axis=1)

    W16 = W.astype(np.float16)
    WT16 = np.ascontiguousarray(W.T).astype(np.float16)
    acol2 = np.concatenate([a[:HD], a[HD:]], axis=1).astype(np.float16)
    shared = {
        "xlocT16": dev_layout(np.ascontiguousarray(x_loc.T).astype(np.float16)),
        "xloc16": dev_layout(x_loc.astype(np.float16)),
    }

    in_maps = []
    for c in range(NCORES):
        b, r = c // 2, c % 2
        rot = r * 512
        u0 = np.concatenate([mob[b, 0, :, 0], text[b, 0, :, 0]]).astype(np.int64)
        l0 = np.concatenate([mob[b, 0, :, 1], text[b, 0, :, 1]]).astype(np.int64)
        cnt = np.bincount(u0, minlength=N_USER).astype(np.float32)
        A = np.zeros((N_USER, N_LOC), np.float32)
        np.add.at(A, (u0, l0), 1.0)
        Ahat = A / np.maximum(cnt, 1.0)[:, None]
        Mb = np.zeros((N_USER, N_LOC), np.float32)
        Tb = np.zeros((N_USER, N_LOC), np.float32)
        Mb[mob[b, 0, :, 0], mob[b, 0, :, 1]] = 1.0
        Tb[text[b, 0, :, 0], text[b, 0, :, 1]] = 1.0
        M = Mb + Tb
        has0 = (cnt > 0).astype(np.float32)
        n_with = max(float(has0.sum()), 1.0)
        nh_cnt = float(N_USER) - float(has0.sum())
        hw = has0 / n_with

        w_mwe = (hw[:, None] * Ahat).sum(0)
        w_sxu = Ahat.sum(0) + nh_cnt * w_mwe
        w_h0 = (w_sxu + 2.0) / 3072.0

        ne = np.zeros(3, np.float32)
        for dd in range(3):
            us = np.concatenate([mob[b, dd + 1, :, 0], text[b, dd + 1, :, 0]])
            hasE = np.zeros(N_USER, np.float32)
            hasE[us] = 1.0
            ne[dd] = hasE.sum()
        nec = np.zeros(6, np.float32)
        for dd in range(3):
            nec[2 * dd] = ne[dd] / 2048.0
            nec[2 * dd + 1] = (N_USER - ne[dd]) / 3072.0

        def rollu(x, axis=0):
            return np.roll(x, -rot, axis=axis)

        wcols = np.stack([w_h0, w_mwe], axis=1).astype(np.float16)
        blob = np.concatenate(
            [dev_layout(W16), dev_layout(WT16), dev_layout(acol2),
             dev_layout(wcols)], axis=1)
        m = dict(shared)
        m.update({
            "blob": np.ascontiguousarray(blob),
            "Ahat8": dev_layout(np.ascontiguousarray(rollu(Ahat, 0).T)).astype(f8),
            "MT16": dev_layout(np.ascontiguousarray(rollu(M, 0).T)).astype(np.float16),
            "nothas16": rollu(1.0 - has0)[None, :].astype(np.float16),
            "necol": np.tile(nec[None, :], (P, 1)).astype(np.float32),
        })
        in_maps.append(m)
    return in_maps


def kernel(**inputs):
    from concourse.bass_utils import run_bass_kernel_spmd

    if "nc" not in _CACHE:
        _CACHE["nc"] = _build_nc()
    nc = _CACHE["nc"]

    x_loc = np.ascontiguousarray(inputs["x_loc"], np.float32)
    mob = np.asarray(inputs["mob_links"])
    text = np.asarray(inputs["text_links"])

    in_maps = _host_prep(inputs["x_loc"], inputs["mob_links"],
                         inputs["text_links"], inputs["W"], inputs["a"])
    res = run_bass_kernel_spmd(nc, in_maps, core_ids=list(range(NCORES)))

    out = np.zeros((B, D, N_USER + 2 * N_LOC, HD), np.float32)
    for c in range(NCORES):
        b, r = c // 2, c % 2
        o0 = np.asarray(res.results[c]["o0"], np.float32)
        o1 = np.asarray(res.results[c]["o1"], np.float32)
        out[b, 0, r * 512:(r + 1) * 512] = (
            o0.reshape(P, 4, HD).transpose(1, 0, 2).reshape(512, HD))
        out[b, 1, r * 512:(r + 1) * 512] = (
            o1.reshape(P, 4, HD).transpose(1, 0, 2).reshape(512, HD))
        if r == 0:
            rows = np.asarray(res.results[c]["rows"], np.float32)
            out[b, 0, N_USER:N_USER + N_LOC] = x_loc
            out[b, 0, N_USER + N_LOC:] = x_loc
            v1 = np.concatenate([rows[:, 0], rows[:, 1]])
            out[b, 1, N_USER:] = v1[None, :]
            for dd, d in enumerate((2, 3, 4)):
                s = 2 + 4 * dd
                r1 = np.concatenate([rows[:, s], rows[:, s + 1]])
                vn = np.concatenate([rows[:, s + 2], rows[:, s + 3]])
                us = np.concatenate([mob[b, d - 1, :, 0], text[b, d - 1, :, 0]])
                hasE = np.zeros(N_USER, bool)
                hasE[us] = True
                out[b, d, :N_USER] = np.where(hasE[:, None], r1[None, :],
                                              vn[None, :])
                out[b, d, N_USER:] = vn[None, :]
    return out


# revision 16
# speedup vs baseline: 2.1988x; 1.0027x over previous
"""Trainium2 Bass kernel for nn_BiGNN_53772990546511.

Math (validated vs reference in mathcheck.py, global l2 rel ~2.5e-4):
  - relu(elu(x)) == relu(x).
  - Day-1 attention collapses to users x locs with multiplicity mask M.
    Softmax is invariant to per-column scaling, so
      exp(leaky(f1_u + f2_l)) ~ max(g_l, t_u) scaled per-row by c_l,
    with g = exp(0.8 f2 + B), t = exp(-0.8 f1 + B), c = exp(0.2 f2 + G).
    The c_l row factor is folded into the whext rows (including the ones
    column), so PT = max(g_l, t_u) * M needs only 2 elementwise ops/tile.
  - Days 2..4 are rank-2 row algebra kept in column form on device (no
    transposes); only 10 row-vectors leave the device and the host
    broadcasts them into the full output (pure gather/unshard).
  - All global sums over users (mwe, mean h0) are rank-1: host prepares
    index-derived weight vectors (w_mwe, w_h0) so phase-2 only computes
    the 512 own-user rows.
  - Device writes per core: 0.5MB (day0+day1 own user halves, f16) +
    rows; everything else is broadcast host-side from row vectors.

Sharding: 8 cores = 4 batches x 2 user-halves.  Odd cores get index
tensors rotated by 512 along users so the program is SPMD-uniform.
"""
import numpy as np

N_USER = 1024
N_LOC = 1024
DM = 256
HD = 256
B = 4
D = 5
E = 4096
ALPHA = 0.2
BETA = -1.0
GAMMA = -0.5
P = 128
NCORES = 8

_CACHE = {}


# --------------------------------------------------------------------------
# Workarounds for this walrus build's 1-sync-wait-per-instruction limit.
# --------------------------------------------------------------------------
def _apply_tile_patch():
    import concourse.tile as tile
    from concourse.tile_sem_assignment import tick_to_sem

    if not getattr(tile.TileContext, "_drain_patched", False):
        def _patched(self, tick_clock, wait_clock):
            nc = self.nc
            gc = tick_clock.global_clock
            for proc, sem in self.sems.allocated().items():
                t = gc[proc]
                if t and t > 0:
                    nc.sync.nop().wait_op(sem, tick_to_sem(t, proc), "sem-ge")
            nc.sync.drain()
            nc.all_engine_barrier()
            popped = nc._tile_sem_poison_stack.pop()
            assert popped is self._sem_poison
            nc.clear_and_free_semaphores(list(self.sems.allocated().values()))
            nc.all_engine_barrier()

        tile.TileContext._drain_and_barrier = _patched
        tile.TileContext._drain_patched = True

    import json as _json
    import concourse.bass_utils as _bu
    import concourse.bass2jax as _b2j

    if not getattr(_bu, "_wait_split_patched", False):
        _orig_compile = _bu.compile_bir_kernel

        def _split_waits(bir_json):
            j = _json.loads(bir_json)
            nid = [0]
            for fn in j.get("functions", []):
                for bb in fn.get("blocks", []):
                    out = []
                    for inst in bb.get("instructions", []):
                        si = inst.get("sync_info") or {}
                        ow = si.get("on_wait") or []
                        if len(ow) > 1:
                            for w in ow[:-1]:
                                nid[0] += 1
                                out.append({
                                    "debug": inst.get("debug", 0),
                                    "engine": inst.get("engine", "SP"),
                                    "ins": [],
                                    "name": f"WSPL-{nid[0]}",
                                    "opcode": "NoOp",
                                    "outs": [],
                                    "sync_info": {"on_update": [],
                                                  "on_wait": [w]},
                                })
                            si["on_wait"] = [ow[-1]]
                        out.append(inst)
                    bb["instructions"] = out
            return _json.dumps(j).encode()

        def _patched_compile(bir_json, tmpdir, neff_name="file.neff"):
            return _orig_compile(_split_waits(bir_json), tmpdir,
                                 neff_name=neff_name)

        _bu.compile_bir_kernel = _patched_compile
        _b2j.compile_bir_kernel = _patched_compile
        _bu._wait_split_patched = True


def _build_nc():
    import contextlib
    import concourse.bass as bass
    import concourse.tile as tile
    from concourse import mybir

    _apply_tile_patch()
    f32 = mybir.dt.float32
    f16 = mybir.dt.float16
    f8 = mybir.dt.float8e4
    AF = mybir.ActivationFunctionType
    OP = mybir.AluOpType

    nc = bass.Bass()

    # ---------------- DRAM tensors (device layout: [128, cols]) ----------------
    d_blob = nc.dram_tensor("blob", [P, 1044], f16, kind="ExternalInput")
    d_nothas = nc.dram_tensor("nothas16", [1, N_USER], f16, kind="ExternalInput")
    d_necol = nc.dram_tensor("necol", [P, 6], f32, kind="ExternalInput")
    d_xlocT = nc.dram_tensor("xlocT16", [P, 2 * N_LOC], f16, kind="ExternalInput")
    d_xloc = nc.dram_tensor("xloc16", [P, 8 * DM], f16, kind="ExternalInput")
    d_A8 = nc.dram_tensor("Ahat8", [P, 8 * N_USER], f8, kind="ExternalInput")
    d_MT = nc.dram_tensor("MT16", [P, 8 * N_USER], f16, kind="ExternalInput")
    d_o0 = nc.dram_tensor("o0", [P, 4 * HD], f16, kind="ExternalOutput")
    d_o1 = nc.dram_tensor("o1", [P, 4 * HD], f16, kind="ExternalOutput")
    d_rows = nc.dram_tensor("rows", [P, 14], f32, kind="ExternalOutput")

    with tile.TileContext(nc) as tc:
        with contextlib.ExitStack() as ctx:
            persist = ctx.enter_context(tc.tile_pool(name="persist", bufs=1))
            work = ctx.enter_context(tc.tile_pool(name="work", bufs=1))
            psBig = ctx.enter_context(tc.tile_pool(name="psBig", bufs=2, space="PSUM"))
            ps6 = ctx.enter_context(tc.tile_pool(name="ps6", bufs=4, space="PSUM"))
            psSm = psBig
            psCol = psBig

            # ------------- input loads, in arrival-priority order -------------
            blob = persist.tile([P, 1044], f16, name="blob")
            nc.sync.dma_start(out=blob[:], in_=d_blob[:])
            Wsb = blob[:, 0:512]
            WTsb = blob[:, 512:1024]
            acol = blob[:, 1024:1028]
            wcol = blob[:, 1028:1044]
            nothas = persist.tile([1, N_USER], f16, name="nothas")
            nc.sync.dma_start(out=nothas[:], in_=d_nothas[:])
            necol = persist.tile([P, 6], f32, name="necol")
            nc.sync.dma_start(out=necol[:], in_=d_necol[:])
            xlocT = persist.tile([P, 2 * N_LOC], f16, name="xlocT")
            nc.sync.dma_start(out=xlocT[:], in_=d_xlocT[:])
            A8 = persist.tile([P, 8 * N_USER], f8, name="A8")
            nc.sync.dma_start(out=A8[:], in_=d_A8[:])
            xloc = persist.tile([P, 8 * DM], f16, name="xloc")
            nc.sync.dma_start(out=xloc[:], in_=d_xloc[:])
            MT = persist.tile([P, 8 * N_USER], f16, name="MT")
            nc.sync.dma_start(out=MT[:, 0:4 * N_USER], in_=d_MT[:, 0:4 * N_USER])
            nc.sync.dma_start(out=MT[:, 4 * N_USER:], in_=d_MT[:, 4 * N_USER:])

            def Wk(kt):
                return Wsb[:, kt * HD:(kt + 1) * HD]

            def WTk(kt):
                return WTsb[:, kt * DM:(kt + 1) * DM]

            def xT(kt, sl):
                s = slice(kt * N_LOC + sl.start, kt * N_LOC + sl.stop)
                return xlocT[:, s]

            def xl(lt):
                return xloc[:, lt * DM:(lt + 1) * DM]

            def Ah(lt, sl):
                s = slice(lt * N_USER + sl.start, lt * N_USER + sl.stop)
                return A8[:, s]

            def Mk(lt):
                return MT[:, lt * N_USER:(lt + 1) * N_USER]

            # constants
            ones16r = persist.tile([1, P], f16, name="ones16r")
            nc.vector.memset(ones16r[:], 1.0)
            one11 = persist.tile([1, 1], f16, name="one11")
            nc.vector.memset(one11[:], 1.0)
            ones16c = persist.tile([P, 1], f16, name="ones16c")
            nc.vector.memset(ones16c[:], 1.0)
            cBETA = persist.tile([P, 1], f32, name="cBETA")
            nc.vector.memset(cBETA[:], BETA)
            cGAMMA = persist.tile([P, 1], f32, name="cGAMMA")
            nc.vector.memset(cGAMMA[:], GAMMA)

            # persistent intermediates
            wa12 = [persist.tile([P, 2], f16, name=f"wa12_{kt}") for kt in range(2)]
            xw1c = persist.tile([P, 8], f16, name="xw1c")
            xw2c = persist.tile([P, 8], f32, name="xw2c")
            g32 = persist.tile([P, 8], f32, name="g32")
            c32 = persist.tile([P, 8], f32, name="c32")
            whext = [persist.tile([P, HD + 1], f16, name=f"whext{lt}")
                     for lt in range(8)]
            mh0row = persist.tile([1, DM], f16, name="mh0row")
            mwe16 = persist.tile([1, DM], f16, name="mwe16")
            mh0c = persist.tile([P, 2], f16, name="mh0c")
            mw0e = persist.tile([1, HD + 1], f16, name="mw0e")
            t16 = persist.tile([1, N_USER], f16, name="t16")
            Tb16 = persist.tile([P, N_USER], f16, name="Tb16")
            PT = persist.tile([P, 8 * N_USER], f16, name="PT")
            day0 = persist.tile([P, 4 * HD], f16, name="day0")
            h1u = persist.tile([P, 8 * HD], f16, name="h1u")
            rows_out = persist.tile([P, 14], f32, name="rows_out")
            rcol = [persist.tile([P, 1], f32, name=f"rcol{ut}") for ut in range(8)]

            # ---------------- wa12 = [W a1 | W a2] as cols ----------------
            for kt in range(2):
                ps = psSm.tile([P, 2], f32, name="psm", tag="sm", bufs=2)
                for ht in range(2):
                    nc.tensor.matmul(ps[:], WTsb[:, ht * DM + kt * P:ht * DM + (kt + 1) * P],
                                     acol[:, ht * 2:ht * 2 + 2],
                                     start=(ht == 0), stop=(ht == 1))
                nc.scalar.activation(wa12[kt][:], ps[:], AF.Copy)

            # ---------------- xw12 cols per loc tile ----------------
            for lt in range(8):
                ps = psSm.tile([P, 2], f32, name="psm", tag="sm", bufs=2)
                for kt in range(2):
                    nc.tensor.matmul(ps[:], xT(kt, slice(lt * P, (lt + 1) * P)),
                                     wa12[kt][:], start=(kt == 0), stop=(kt == 1))
                nc.vector.tensor_copy(xw1c[:, lt:lt + 1], ps[:, 0:1])
                nc.vector.tensor_copy(xw2c[:, lt:lt + 1], ps[:, 1:2])
            nc.scalar.activation(g32[:], xw2c[:], AF.Exp, scale=0.8, bias=cBETA[:])
            nc.scalar.activation(c32[:], xw2c[:], AF.Exp, scale=0.2, bias=cGAMMA[:])

            # ---------------- f1row -> t -> Tb ----------------
            for ch in range(2):
                csl = slice(ch * 512, (ch + 1) * 512)
                pf = psBig.tile([1, 512], f32, name="pf1", tag="big")
                for lt in range(8):
                    nc.tensor.matmul(pf[:], xw1c[:, lt:lt + 1], Ah(lt, csl),
                                     start=(lt == 0), stop=(lt == 7))
                nc.scalar.activation(t16[0:1, csl], pf[:], AF.Exp,
                                     scale=-0.8, bias=cBETA[0:1, :])
                ptb = psBig.tile([P, 512], f32, name="ptb", tag="big")
                nc.tensor.matmul(ptb[:], ones16r[:], t16[0:1, csl],
                                 start=True, stop=True)
                nc.scalar.activation(Tb16[:, csl], ptb[:], AF.Copy)

            # ---------------- whext_c = c_l * [Wh_l | 1] ----------------
            for lt in range(8):
                ps = psBig.tile([P, HD], f32, name="pwh", tag="big")
                for kt in range(2):
                    nc.tensor.matmul(ps[:], xT(kt, slice(lt * P, (lt + 1) * P)),
                                     Wk(kt), start=(kt == 0), stop=(kt == 1))
                nc.scalar.activation(whext[lt][:, 0:HD], ps[:], AF.Copy,
                                     scale=c32[:, lt:lt + 1])
                nc.vector.tensor_copy(whext[lt][:, HD:HD + 1], c32[:, lt:lt + 1])

            # ---------------- mh0 / mwe rows ----------------
            pmr = psBig.tile([1, DM], f32, name="pmr", tag="big")
            for lt in range(8):
                nc.tensor.matmul(pmr[:], wcol[:, lt * 2:lt * 2 + 1], xl(lt),
                                 start=(lt == 0), stop=(lt == 7))
            nc.scalar.activation(mh0row[:], pmr[:], AF.Copy)
            pmr2 = psBig.tile([1, DM], f32, name="pmr2", tag="big")
            for lt in range(8):
                nc.tensor.matmul(pmr2[:], wcol[:, lt * 2 + 1:lt * 2 + 2], xl(lt),
                                 start=(lt == 0), stop=(lt == 7))
            nc.scalar.activation(mwe16[:], pmr2[:], AF.Copy)
            for mt in range(2):
                ps = psSm.tile([P, 1], f32, name="psm", tag="sm", bufs=2)
                nc.tensor.matmul(ps[:], mh0row[0:1, mt * P:(mt + 1) * P],
                                 one11[:], start=True, stop=True)
                nc.vector.tensor_copy(mh0c[:, mt:mt + 1], ps[:])
            pmw = psBig.tile([1, HD], f32, name="pmw", tag="big")
            for kt in range(2):
                nc.tensor.matmul(pmw[:], mh0c[:, kt:kt + 1], Wk(kt),
                                 start=(kt == 0), stop=(kt == 1))
            nc.scalar.activation(mw0e[:, 0:HD], pmw[:], AF.Copy)
            nc.vector.memset(mw0e[:, HD:HD + 1], 1.0)

            # mw0 cols -> v1 (day-1 loc row) and vs-chain init
            vs1 = [work.tile([P, 3], f16, name=f"vs1_{kt}", tag=f"vs1_{kt}")
                   for kt in range(2)]
            for mt in range(2):
                ps = psCol.tile([P, 1], f32, name="pcol", tag="sm", bufs=2)
                for kt in range(2):
                    nc.tensor.matmul(ps[:], Wsb[:, kt * HD + mt * P:kt * HD + (mt + 1) * P],
                                     mh0c[:, kt:kt + 1],
                                     start=(kt == 0), stop=(kt == 1))
                nc.scalar.activation(rows_out[:, mt:mt + 1], ps[:], AF.Relu)
                nc.scalar.activation(vs1[mt][:, 0:1], ps[:], AF.Relu,
                                     scale=2048.0 / 3072.0)

            # ---------------- phase 2: x_user own half ----------------
            for ut in range(4):
                ps = psBig.tile([P, DM], f32, name="px", tag="big")
                for lt in range(8):
                    nc.tensor.matmul(ps[:], Ah(lt, slice(ut * P, (ut + 1) * P)),
                                     xl(lt), start=(lt == 0), stop=False)
                nc.tensor.matmul(ps[:], nothas[0:1, ut * P:(ut + 1) * P],
                                 mwe16[:], start=False, stop=True)
                nc.scalar.activation(day0[:, ut * HD:(ut + 1) * HD], ps[:], AF.Copy)
            nc.scalar.dma_start(out=d_o0[:, :], in_=day0[:])

            # ---------------- PT + phase 6 (interleaved over loc tiles) -----
            for half in range(2):
                tiles = [ps6.tile([P, HD + 1], f32, name=f"p6_{half}_{i}",
                                  tag="p6") for i in range(4)]
                for lt in range(8):
                    csl = slice(lt * N_USER + half * 512,
                                lt * N_USER + half * 512 + 512)
                    tsl = slice(half * 512, half * 512 + 512)
                    nc.vector.scalar_tensor_tensor(
                        out=PT[:, csl], in0=Tb16[:, tsl],
                        scalar=g32[:, lt:lt + 1], in1=MT[:, csl],
                        op0=OP.max, op1=OP.mult)
                    for i in range(4):
                        ut = half * 4 + i
                        usl = slice(lt * N_USER + ut * P,
                                    lt * N_USER + (ut + 1) * P)
                        nc.tensor.matmul(tiles[i][:], PT[:, usl], whext[lt][:],
                                         start=(lt == 0), stop=False)
                for i in range(4):
                    ut = half * 4 + i
                    nc.tensor.matmul(tiles[i][:],
                                     nothas[0:1, ut * P:(ut + 1) * P],
                                     mw0e[:], start=False, stop=True)
                    nc.vector.reciprocal(rcol[ut][:], tiles[i][:, HD:HD + 1])
                    nc.scalar.activation(h1u[:, ut * HD:(ut + 1) * HD],
                                         tiles[i][:, 0:HD], AF.Relu,
                                         scale=rcol[ut][:])
                if half == 0:
                    nc.scalar.dma_start(out=d_o1[:, :], in_=h1u[:, 0:4 * HD])

            # ---------------- scol ----------------
            for mt in range(2):
                ps = psCol.tile([P, 1], f32, name="pcol", tag="sm", bufs=2)
                for ut in range(8):
                    nc.tensor.matmul(
                        ps[:], h1u[:, ut * HD + mt * P:ut * HD + (mt + 1) * P],
                        ones16c[:], start=(ut == 0), stop=(ut == 7))
                nc.scalar.activation(vs1[mt][:, 1:2], ps[:], AF.Copy,
                                     scale=1.0 / 3072.0)
                nc.vector.tensor_tensor(out=vs1[mt][:, 2:3], in0=vs1[mt][:, 0:1],
                                        in1=vs1[mt][:, 1:2], op=OP.add)

            # ---------------- days 2..4 ----------------
            vs_cur = vs1
            for dd in range(3):
                base = 2 + 4 * dd
                vs_n = [work.tile([P, 3], f16, name=f"vs{dd}_{kt}",
                                  tag=f"vs{dd}_{kt}") for kt in range(2)]
                for mt in range(2):
                    ps = psSm.tile([P, 3], f32, name="psm", tag="sm", bufs=2)
                    for kt in range(2):
                        nc.tensor.matmul(ps[:], Wsb[:, kt * HD + mt * P:kt * HD + (mt + 1) * P],
                                         vs_cur[kt][:],
                                         start=(kt == 0), stop=(kt == 1))
                    nc.scalar.activation(rows_out[:, base + mt:base + mt + 1],
                                         ps[:, 0:1], AF.Relu, scale=1.5)
                    nc.scalar.activation(rows_out[:, base + 2 + mt:base + 3 + mt],
                                         ps[:, 2:3], AF.Relu)
                    if dd < 2:
                        nc.scalar.activation(vs_n[mt][:, 0:1], ps[:, 2:3],
                                             AF.Relu, scale=2048.0 / 3072.0)
                        tA = work.tile([P, 1], f32, name=f"tA{dd}_{mt}",
                                       tag=f"tA{dd}_{mt}")
                        tB = work.tile([P, 1], f32, name=f"tB{dd}_{mt}",
                                       tag=f"tB{dd}_{mt}")
                        nc.scalar.activation(tA[:], ps[:, 0:1], AF.Relu,
                                             scale=necol[:, 2 * dd:2 * dd + 1])
                        nc.scalar.activation(tB[:], ps[:, 2:3], AF.Relu,
                                             scale=necol[:, 2 * dd + 1:2 * dd + 2])
                        nc.vector.tensor_tensor(out=vs_n[mt][:, 1:2], in0=tA[:],
                                                in1=tB[:], op=OP.add)
                        nc.vector.tensor_tensor(out=vs_n[mt][:, 2:3],
                                                in0=vs_n[mt][:, 0:1],
                                                in1=vs_n[mt][:, 1:2], op=OP.add)
                vs_cur = vs_n
            nc.scalar.dma_start(out=d_rows[:, :], in_=rows_out[:])

    return nc


def _host_prep(x_loc, mob_links, text_links, W, a):
    """Index-derived preprocessing -> per-core input maps."""
    import ml_dtypes
    f8 = ml_dtypes.float8_e4m3

    x_loc = np.ascontiguousarray(x_loc, np.float32)
    W = np.ascontiguousarray(W, np.float32)
    a = np.ascontiguousarray(a, np.float32)
    mob = np.asarray(mob_links)
    text = np.asarray(text_links)

    def dev_layout(x):
        t = x.shape[0] // P
        return np.concatenate([x[i * P:(i + 1) * P] for i in range(t)], axis=1)

    W16 = W.astype(np.float16)
    WT16 = np.ascontiguousarray(W.T).astype(np.float16)
    acol2 = np.concatenate([a[:HD], a[HD:]], axis=1).astype(np.float16)
    shared = {
        "xlocT16": dev_layout(np.ascontiguousarray(x_loc.T).astype(np.float16)),
        "xloc16": dev_layout(x_loc.astype(np.float16)),
    }

    in_maps = []
    for c in range(NCORES):
        b, r = c // 2, c % 2
        rot = r * 512
        u0 = np.concatenate([mob[b, 0, :, 0], text[b, 0, :, 0]]).astype(np.int64)
        l0 = np.concatenate([mob[b, 0, :, 1], text[b, 0, :, 1]]).astype(np.int64)
        cnt = np.bincount(u0, minlength=N_USER).astype(np.float32)
        A = np.zeros((N_USER, N_LOC), np.float32)
        np.add.at(A, (u0, l0), 1.0)
        Ahat = A / np.maximum(cnt, 1.0)[:, None]
        Mb = np.zeros((N_USER, N_LOC), np.float32)
        Tb = np.zeros((N_USER, N_LOC), np.float32)
        Mb[mob[b, 0, :, 0], mob[b, 0, :, 1]] = 1.0
        Tb[text[b, 0, :, 0], text[b, 0, :, 1]] = 1.0
        M = Mb + Tb
        has0 = (cnt > 0).astype(np.float32)
        n_with = max(float(has0.sum()), 1.0)
        nh_cnt = float(N_USER) - float(has0.sum())
        hw = has0 / n_with

        w_mwe = (hw[:, None] * Ahat).sum(0)
        w_sxu = Ahat.sum(0) + nh_cnt * w_mwe
        w_h0 = (w_sxu + 2.0) / 3072.0

        ne = np.zeros(3, np.float32)
        for dd in range(3):
            us = np.concatenate([mob[b, dd + 1, :, 0], text[b, dd + 1, :, 0]])
            hasE = np.zeros(N_USER, np.float32)
            hasE[us] = 1.0
            ne[dd] = hasE.sum()
        nec = np.zeros(6, np.float32)
        for dd in range(3):
            nec[2 * dd] = ne[dd] / 2048.0
            nec[2 * dd + 1] = (N_USER - ne[dd]) / 3072.0

        def rollu(x, axis=0):
            return np.roll(x, -rot, axis=axis)

        wcols = np.stack([w_h0, w_mwe], axis=1).astype(np.float16)
        blob = np.concatenate(
            [dev_layout(W16), dev_layout(WT16), dev_layout(acol2),
             dev_layout(wcols)], axis=1)
        m = dict(shared)
        m.update({
            "blob": np.ascontiguousarray(blob),
            "Ahat8": dev_layout(np.ascontiguousarray(rollu(Ahat, 0).T)).astype(f8),
            "MT16": dev_layout(np.ascontiguousarray(rollu(M, 0).T)).astype(np.float16),
            "nothas16": rollu(1.0 - has0)[None, :].astype(np.float16),
            "necol": np.tile(nec[None, :], (P, 1)).astype(np.float32),
        })
        in_maps.append(m)
    return in_maps


def kernel(**inputs):
    from concourse.bass_utils import run_bass_kernel_spmd

    if "nc" not in _CACHE:
        _CACHE["nc"] = _build_nc()
    nc = _CACHE["nc"]

    x_loc = np.ascontiguousarray(inputs["x_loc"], np.float32)
    mob = np.asarray(inputs["mob_links"])
    text = np.asarray(inputs["text_links"])

    in_maps = _host_prep(inputs["x_loc"], inputs["mob_links"],
                         inputs["text_links"], inputs["W"], inputs["a"])
    res = run_bass_kernel_spmd(nc, in_maps, core_ids=list(range(NCORES)))

    out = np.zeros((B, D, N_USER + 2 * N_LOC, HD), np.float32)
    for c in range(NCORES):
        b, r = c // 2, c % 2
        o0 = np.asarray(res.results[c]["o0"], np.float32)
        o1 = np.asarray(res.results[c]["o1"], np.float32)
        out[b, 0, r * 512:(r + 1) * 512] = (
            o0.reshape(P, 4, HD).transpose(1, 0, 2).reshape(512, HD))
        out[b, 1, r * 512:(r + 1) * 512] = (
            o1.reshape(P, 4, HD).transpose(1, 0, 2).reshape(512, HD))
        if r == 0:
            rows = np.asarray(res.results[c]["rows"], np.float32)
            out[b, 0, N_USER:N_USER + N_LOC] = x_loc
            out[b, 0, N_USER + N_LOC:] = x_loc
            v1 = np.concatenate([rows[:, 0], rows[:, 1]])
            out[b, 1, N_USER:] = v1[None, :]
            for dd, d in enumerate((2, 3, 4)):
                s = 2 + 4 * dd
                r1 = np.concatenate([rows[:, s], rows[:, s + 1]])
                vn = np.concatenate([rows[:, s + 2], rows[:, s + 3]])
                us = np.concatenate([mob[b, d - 1, :, 0], text[b, d - 1, :, 0]])
                hasE = np.zeros(N_USER, bool)
                hasE[us] = True
                out[b, d, :N_USER] = np.where(hasE[:, None], r1[None, :],
                                              vn[None, :])
                out[b, d, N_USER:] = vn[None, :]
    return out


# revision 17
# speedup vs baseline: 2.2544x; 1.0253x over previous
"""Trainium2 Bass kernel for nn_BiGNN_53772990546511.

Math (validated vs reference in mathcheck.py, global l2 rel ~2.5e-4):
  - relu(elu(x)) == relu(x).
  - Day-1 attention collapses to users x locs with multiplicity mask M.
    Softmax is invariant to per-column scaling, so
      exp(leaky(f1_u + f2_l)) ~ max(g_l, t_u) scaled per-row by c_l,
    with g = exp(0.8 f2 + B), t = exp(-0.8 f1 + B), c = exp(0.2 f2 + G).
    The c_l row factor is folded into the whext rows (including the ones
    column), so PT = max(g_l, t_u) * M needs only 2 elementwise ops/tile.
  - Days 2..4 are rank-2 row algebra kept in column form on device (no
    transposes); only 10 row-vectors leave the device and the host
    broadcasts them into the full output (pure gather/unshard).
  - All global sums over users (mwe, mean h0) are rank-1: host prepares
    index-derived weight vectors (w_mwe, w_h0) so phase-2 only computes
    the 512 own-user rows.
  - Device writes per core: 0.5MB (day0+day1 own user halves, f16) +
    rows; everything else is broadcast host-side from row vectors.

Sharding: 8 cores = 4 batches x 2 user-halves.  Odd cores get index
tensors rotated by 512 along users so the program is SPMD-uniform.
"""
import numpy as np

N_USER = 1024
N_LOC = 1024
DM = 256
HD = 256
B = 4
D = 5
E = 4096
ALPHA = 0.2
BETA = -1.0
GAMMA = -0.5
P = 128
NCORES = 8

_CACHE = {}


# --------------------------------------------------------------------------
# Workarounds for this walrus build's 1-sync-wait-per-instruction limit.
# --------------------------------------------------------------------------
def _apply_tile_patch():
    import concourse.tile as tile
    from concourse.tile_sem_assignment import tick_to_sem

    if not getattr(tile.TileContext, "_drain_patched", False):
        def _patched(self, tick_clock, wait_clock):
            nc = self.nc
            gc = tick_clock.global_clock
            for proc, sem in self.sems.allocated().items():
                t = gc[proc]
                if t and t > 0:
                    nc.sync.nop().wait_op(sem, tick_to_sem(t, proc), "sem-ge")
            nc.sync.drain()
            nc.all_engine_barrier()
            popped = nc._tile_sem_poison_stack.pop()
            assert popped is self._sem_poison
            nc.clear_and_free_semaphores(list(self.sems.allocated().values()))
            nc.all_engine_barrier()

        tile.TileContext._drain_and_barrier = _patched
        tile.TileContext._drain_patched = True

    import json as _json
    import concourse.bass_utils as _bu
    import concourse.bass2jax as _b2j

    if not getattr(_bu, "_wait_split_patched", False):
        _orig_compile = _bu.compile_bir_kernel

        def _split_waits(bir_json):
            j = _json.loads(bir_json)
            nid = [0]
            for fn in j.get("functions", []):
                for bb in fn.get("blocks", []):
                    out = []
                    for inst in bb.get("instructions", []):
                        si = inst.get("sync_info") or {}
                        ow = si.get("on_wait") or []
                        if len(ow) > 1:
                            for w in ow[:-1]:
                                nid[0] += 1
                                out.append({
                                    "debug": inst.get("debug", 0),
                                    "engine": inst.get("engine", "SP"),
                                    "ins": [],
                                    "name": f"WSPL-{nid[0]}",
                                    "opcode": "NoOp",
                                    "outs": [],
                                    "sync_info": {"on_update": [],
                                                  "on_wait": [w]},
                                })
                            si["on_wait"] = [ow[-1]]
                        out.append(inst)
                    bb["instructions"] = out
            return _json.dumps(j).encode()

        def _patched_compile(bir_json, tmpdir, neff_name="file.neff"):
            return _orig_compile(_split_waits(bir_json), tmpdir,
                                 neff_name=neff_name)

        _bu.compile_bir_kernel = _patched_compile
        _b2j.compile_bir_kernel = _patched_compile
        _bu._wait_split_patched = True


def _build_nc():
    import contextlib
    import concourse.bass as bass
    import concourse.tile as tile
    from concourse import mybir

    _apply_tile_patch()
    f32 = mybir.dt.float32
    f16 = mybir.dt.float16
    f8 = mybir.dt.float8e4
    AF = mybir.ActivationFunctionType
    OP = mybir.AluOpType

    nc = bass.Bass()

    # ---------------- DRAM tensors (device layout: [128, cols]) ----------------
    d_blob = nc.dram_tensor("blob", [P, 1044], f16, kind="ExternalInput")
    d_nothas = nc.dram_tensor("nothas16", [1, N_USER], f16, kind="ExternalInput")
    d_necol = nc.dram_tensor("necol", [P, 6], f32, kind="ExternalInput")
    d_xlocT = nc.dram_tensor("xlocT16", [P, 2 * N_LOC], f16, kind="ExternalInput")
    d_xloc = nc.dram_tensor("xloc16", [P, 8 * DM], f16, kind="ExternalInput")
    d_A8 = nc.dram_tensor("Ahat8", [P, 8 * N_USER], f8, kind="ExternalInput")
    d_MT = nc.dram_tensor("MT16", [P, 8 * N_USER], f8, kind="ExternalInput")
    d_o0 = nc.dram_tensor("o0", [P, 4 * HD], f16, kind="ExternalOutput")
    d_o1 = nc.dram_tensor("o1", [P, 4 * HD], f16, kind="ExternalOutput")
    d_rows = nc.dram_tensor("rows", [P, 14], f32, kind="ExternalOutput")

    with tile.TileContext(nc) as tc:
        with contextlib.ExitStack() as ctx:
            persist = ctx.enter_context(tc.tile_pool(name="persist", bufs=1))
            work = ctx.enter_context(tc.tile_pool(name="work", bufs=1))
            psBig = ctx.enter_context(tc.tile_pool(name="psBig", bufs=2, space="PSUM"))
            ps6 = ctx.enter_context(tc.tile_pool(name="ps6", bufs=4, space="PSUM"))
            psSm = psBig
            psCol = psBig

            # ------------- input loads, in arrival-priority order -------------
            blob = persist.tile([P, 1044], f16, name="blob")
            nc.sync.dma_start(out=blob[:], in_=d_blob[:])
            Wsb = blob[:, 0:512]
            WTsb = blob[:, 512:1024]
            acol = blob[:, 1024:1028]
            wcol = blob[:, 1028:1044]
            nothas = persist.tile([1, N_USER], f16, name="nothas")
            nc.sync.dma_start(out=nothas[:], in_=d_nothas[:])
            necol = persist.tile([P, 6], f32, name="necol")
            nc.sync.dma_start(out=necol[:], in_=d_necol[:])
            xlocT = persist.tile([P, 2 * N_LOC], f16, name="xlocT")
            nc.sync.dma_start(out=xlocT[:], in_=d_xlocT[:])
            A8 = persist.tile([P, 8 * N_USER], f8, name="A8")
            nc.sync.dma_start(out=A8[:], in_=d_A8[:])
            xloc = persist.tile([P, 8 * DM], f16, name="xloc")
            nc.sync.dma_start(out=xloc[:], in_=d_xloc[:])
            MT = persist.tile([P, 8 * N_USER], f8, name="MT")
            nc.sync.dma_start(out=MT[:, 0:4 * N_USER], in_=d_MT[:, 0:4 * N_USER])
            nc.sync.dma_start(out=MT[:, 4 * N_USER:], in_=d_MT[:, 4 * N_USER:])

            def Wk(kt):
                return Wsb[:, kt * HD:(kt + 1) * HD]

            def WTk(kt):
                return WTsb[:, kt * DM:(kt + 1) * DM]

            def xT(kt, sl):
                s = slice(kt * N_LOC + sl.start, kt * N_LOC + sl.stop)
                return xlocT[:, s]

            def xl(lt):
                return xloc[:, lt * DM:(lt + 1) * DM]

            def Ah(lt, sl):
                s = slice(lt * N_USER + sl.start, lt * N_USER + sl.stop)
                return A8[:, s]

            def Mk(lt):
                return MT[:, lt * N_USER:(lt + 1) * N_USER]

            # constants
            ones16r = persist.tile([1, P], f16, name="ones16r")
            nc.vector.memset(ones16r[:], 1.0)
            one11 = persist.tile([1, 1], f16, name="one11")
            nc.vector.memset(one11[:], 1.0)
            ones16c = persist.tile([P, 1], f16, name="ones16c")
            nc.vector.memset(ones16c[:], 1.0)
            cBETA = persist.tile([P, 1], f32, name="cBETA")
            nc.vector.memset(cBETA[:], BETA)
            cGAMMA = persist.tile([P, 1], f32, name="cGAMMA")
            nc.vector.memset(cGAMMA[:], GAMMA)

            # persistent intermediates
            wa12 = [persist.tile([P, 2], f16, name=f"wa12_{kt}") for kt in range(2)]
            xw1c = persist.tile([P, 8], f16, name="xw1c")
            xw2c = persist.tile([P, 8], f32, name="xw2c")
            g32 = persist.tile([P, 8], f32, name="g32")
            c32 = persist.tile([P, 8], f32, name="c32")
            whext = [persist.tile([P, HD + 1], f16, name=f"whext{lt}")
                     for lt in range(8)]
            mh0row = persist.tile([1, DM], f16, name="mh0row")
            mwe16 = persist.tile([1, DM], f16, name="mwe16")
            mh0c = persist.tile([P, 2], f16, name="mh0c")
            mw0e = persist.tile([1, HD + 1], f16, name="mw0e")
            t16 = persist.tile([1, N_USER], f16, name="t16")
            Tb16 = persist.tile([P, N_USER], f16, name="Tb16")
            PT = persist.tile([P, 8 * N_USER], f16, name="PT")
            day0 = persist.tile([P, 4 * HD], f16, name="day0")
            h1u = persist.tile([P, 8 * HD], f16, name="h1u")
            rows_out = persist.tile([P, 14], f32, name="rows_out")
            rcol = [persist.tile([P, 1], f32, name=f"rcol{ut}") for ut in range(8)]

            # ---------------- wa12 = [W a1 | W a2] as cols ----------------
            for kt in range(2):
                ps = psSm.tile([P, 2], f32, name="psm", tag="sm", bufs=2)
                for ht in range(2):
                    nc.tensor.matmul(ps[:], WTsb[:, ht * DM + kt * P:ht * DM + (kt + 1) * P],
                                     acol[:, ht * 2:ht * 2 + 2],
                                     start=(ht == 0), stop=(ht == 1))
                nc.scalar.activation(wa12[kt][:], ps[:], AF.Copy)

            # ---------------- xw12 cols per loc tile ----------------
            for lt in range(8):
                ps = psSm.tile([P, 2], f32, name="psm", tag="sm", bufs=2)
                for kt in range(2):
                    nc.tensor.matmul(ps[:], xT(kt, slice(lt * P, (lt + 1) * P)),
                                     wa12[kt][:], start=(kt == 0), stop=(kt == 1))
                nc.vector.tensor_copy(xw1c[:, lt:lt + 1], ps[:, 0:1])
                nc.vector.tensor_copy(xw2c[:, lt:lt + 1], ps[:, 1:2])
            nc.scalar.activation(g32[:], xw2c[:], AF.Exp, scale=0.8, bias=cBETA[:])
            nc.scalar.activation(c32[:], xw2c[:], AF.Exp, scale=0.2, bias=cGAMMA[:])

            # ---------------- f1row -> t -> Tb ----------------
            for ch in range(2):
                csl = slice(ch * 512, (ch + 1) * 512)
                pf = psBig.tile([1, 512], f32, name="pf1", tag="big")
                for lt in range(8):
                    nc.tensor.matmul(pf[:], xw1c[:, lt:lt + 1], Ah(lt, csl),
                                     start=(lt == 0), stop=(lt == 7))
                nc.scalar.activation(t16[0:1, csl], pf[:], AF.Exp,
                                     scale=-0.8, bias=cBETA[0:1, :])
                ptb = psBig.tile([P, 512], f32, name="ptb", tag="big")
                nc.tensor.matmul(ptb[:], ones16r[:], t16[0:1, csl],
                                 start=True, stop=True)
                nc.scalar.activation(Tb16[:, csl], ptb[:], AF.Copy)

            # ---------------- whext_c = c_l * [Wh_l | 1] ----------------
            for lt in range(8):
                ps = psBig.tile([P, HD], f32, name="pwh", tag="big")
                for kt in range(2):
                    nc.tensor.matmul(ps[:], xT(kt, slice(lt * P, (lt + 1) * P)),
                                     Wk(kt), start=(kt == 0), stop=(kt == 1))
                nc.scalar.activation(whext[lt][:, 0:HD], ps[:], AF.Copy,
                                     scale=c32[:, lt:lt + 1])
                nc.vector.tensor_copy(whext[lt][:, HD:HD + 1], c32[:, lt:lt + 1])

            # ---------------- mh0 / mwe rows ----------------
            pmr = psBig.tile([1, DM], f32, name="pmr", tag="big")
            for lt in range(8):
                nc.tensor.matmul(pmr[:], wcol[:, lt * 2:lt * 2 + 1], xl(lt),
                                 start=(lt == 0), stop=(lt == 7))
            nc.scalar.activation(mh0row[:], pmr[:], AF.Copy)
            pmr2 = psBig.tile([1, DM], f32, name="pmr2", tag="big")
            for lt in range(8):
                nc.tensor.matmul(pmr2[:], wcol[:, lt * 2 + 1:lt * 2 + 2], xl(lt),
                                 start=(lt == 0), stop=(lt == 7))
            nc.scalar.activation(mwe16[:], pmr2[:], AF.Copy)
            for mt in range(2):
                ps = psSm.tile([P, 1], f32, name="psm", tag="sm", bufs=2)
                nc.tensor.matmul(ps[:], mh0row[0:1, mt * P:(mt + 1) * P],
                                 one11[:], start=True, stop=True)
                nc.vector.tensor_copy(mh0c[:, mt:mt + 1], ps[:])
            pmw = psBig.tile([1, HD], f32, name="pmw", tag="big")
            for kt in range(2):
                nc.tensor.matmul(pmw[:], mh0c[:, kt:kt + 1], Wk(kt),
                                 start=(kt == 0), stop=(kt == 1))
            nc.scalar.activation(mw0e[:, 0:HD], pmw[:], AF.Copy)
            nc.vector.memset(mw0e[:, HD:HD + 1], 1.0)

            # mw0 cols -> v1 (day-1 loc row) and vs-chain init
            vs1 = [work.tile([P, 3], f16, name=f"vs1_{kt}", tag=f"vs1_{kt}")
                   for kt in range(2)]
            for mt in range(2):
                ps = psCol.tile([P, 1], f32, name="pcol", tag="sm", bufs=2)
                for kt in range(2):
                    nc.tensor.matmul(ps[:], Wsb[:, kt * HD + mt * P:kt * HD + (mt + 1) * P],
                                     mh0c[:, kt:kt + 1],
                                     start=(kt == 0), stop=(kt == 1))
                nc.scalar.activation(rows_out[:, mt:mt + 1], ps[:], AF.Relu)
                nc.scalar.activation(vs1[mt][:, 0:1], ps[:], AF.Relu,
                                     scale=2048.0 / 3072.0)

            # ---------------- phase 2: x_user own half ----------------
            for ut in range(4):
                ps = psBig.tile([P, DM], f32, name="px", tag="big")
                for lt in range(8):
                    nc.tensor.matmul(ps[:], Ah(lt, slice(ut * P, (ut + 1) * P)),
                                     xl(lt), start=(lt == 0), stop=False)
                nc.tensor.matmul(ps[:], nothas[0:1, ut * P:(ut + 1) * P],
                                 mwe16[:], start=False, stop=True)
                nc.scalar.activation(day0[:, ut * HD:(ut + 1) * HD], ps[:], AF.Copy)
            nc.scalar.dma_start(out=d_o0[:, :], in_=day0[:])

            # ---------------- PT + phase 6 (interleaved over loc tiles) -----
            for half in range(2):
                tiles = [ps6.tile([P, HD + 1], f32, name=f"p6_{half}_{i}",
                                  tag="p6") for i in range(4)]
                for lt in range(8):
                    csl = slice(lt * N_USER + half * 512,
                                lt * N_USER + half * 512 + 512)
                    tsl = slice(half * 512, half * 512 + 512)
                    nc.vector.scalar_tensor_tensor(
                        out=PT[:, csl], in0=Tb16[:, tsl],
                        scalar=g32[:, lt:lt + 1], in1=MT[:, csl],
                        op0=OP.max, op1=OP.mult)
                    for i in range(4):
                        ut = half * 4 + i
                        usl = slice(lt * N_USER + ut * P,
                                    lt * N_USER + (ut + 1) * P)
                        nc.tensor.matmul(tiles[i][:], PT[:, usl], whext[lt][:],
                                         start=(lt == 0), stop=False)
                for i in range(4):
                    ut = half * 4 + i
                    nc.tensor.matmul(tiles[i][:],
                                     nothas[0:1, ut * P:(ut + 1) * P],
                                     mw0e[:], start=False, stop=True)
                    nc.vector.reciprocal(rcol[ut][:], tiles[i][:, HD:HD + 1])
                    nc.scalar.activation(h1u[:, ut * HD:(ut + 1) * HD],
                                         tiles[i][:, 0:HD], AF.Relu,
                                         scale=rcol[ut][:])
                if half == 0:
                    nc.scalar.dma_start(out=d_o1[:, :], in_=h1u[:, 0:4 * HD])

            # ---------------- scol ----------------
            for mt in range(2):
                ps = psCol.tile([P, 1], f32, name="pcol", tag="sm", bufs=2)
                for ut in range(8):
                    nc.tensor.matmul(
                        ps[:], h1u[:, ut * HD + mt * P:ut * HD + (mt + 1) * P],
                        ones16c[:], start=(ut == 0), stop=(ut == 7))
                nc.scalar.activation(vs1[mt][:, 1:2], ps[:], AF.Copy,
                                     scale=1.0 / 3072.0)
                nc.vector.tensor_tensor(out=vs1[mt][:, 2:3], in0=vs1[mt][:, 0:1],
                                        in1=vs1[mt][:, 1:2], op=OP.add)

            # ---------------- days 2..4 ----------------
            vs_cur = vs1
            for dd in range(3):
                base = 2 + 4 * dd
                vs_n = [work.tile([P, 3], f16, name=f"vs{dd}_{kt}",
                                  tag=f"vs{dd}_{kt}") for kt in range(2)]
                for mt in range(2):
                    ps = psSm.tile([P, 3], f32, name="psm", tag="sm", bufs=2)
                    for kt in range(2):
                        nc.tensor.matmul(ps[:], Wsb[:, kt * HD + mt * P:kt * HD + (mt + 1) * P],
                                         vs_cur[kt][:],
                                         start=(kt == 0), stop=(kt == 1))
                    nc.scalar.activation(rows_out[:, base + mt:base + mt + 1],
                                         ps[:, 0:1], AF.Relu, scale=1.5)
                    nc.scalar.activation(rows_out[:, base + 2 + mt:base + 3 + mt],
                                         ps[:, 2:3], AF.Relu)
                    if dd < 2:
                        nc.scalar.activation(vs_n[mt][:, 0:1], ps[:, 2:3],
                                             AF.Relu, scale=2048.0 / 3072.0)
                        tA = work.tile([P, 1], f32, name=f"tA{dd}_{mt}",
                                       tag=f"tA{dd}_{mt}")
                        tB = work.tile([P, 1], f32, name=f"tB{dd}_{mt}",
                                       tag=f"tB{dd}_{mt}")
                        nc.scalar.activation(tA[:], ps[:, 0:1], AF.Relu,
                                             scale=necol[:, 2 * dd:2 * dd + 1])
                        nc.scalar.activation(tB[:], ps[:, 2:3], AF.Relu,
                                             scale=necol[:, 2 * dd + 1:2 * dd + 2])
                        nc.vector.tensor_tensor(out=vs_n[mt][:, 1:2], in0=tA[:],
                                                in1=tB[:], op=OP.add)
                        nc.vector.tensor_tensor(out=vs_n[mt][:, 2:3],
                                                in0=vs_n[mt][:, 0:1],
                                                in1=vs_n[mt][:, 1:2], op=OP.add)
                vs_cur = vs_n
            nc.scalar.dma_start(out=d_rows[:, :], in_=rows_out[:])

    return nc


def _host_prep(x_loc, mob_links, text_links, W, a):
    """Index-derived preprocessing -> per-core input maps."""
    import ml_dtypes
    f8 = ml_dtypes.float8_e4m3

    x_loc = np.ascontiguousarray(x_loc, np.float32)
    W = np.ascontiguousarray(W, np.float32)
    a = np.ascontiguousarray(a, np.float32)
    mob = np.asarray(mob_links)
    text = np.asarray(text_links)

    def dev_layout(x):
        t = x.shape[0] // P
        return np.concatenate([x[i * P:(i + 1) * P] for i in range(t)], axis=1)

    W16 = W.astype(np.float16)
    WT16 = np.ascontiguousarray(W.T).astype(np.float16)
    acol2 = np.concatenate([a[:HD], a[HD:]], axis=1).astype(np.float16)
    shared = {
        "xlocT16": dev_layout(np.ascontiguousarray(x_loc.T).astype(np.float16)),
        "xloc16": dev_layout(x_loc.astype(np.float16)),
    }

    in_maps = []
    for c in range(NCORES):
        b, r = c // 2, c % 2
        rot = r * 512
        u0 = np.concatenate([mob[b, 0, :, 0], text[b, 0, :, 0]]).astype(np.int64)
        l0 = np.concatenate([mob[b, 0, :, 1], text[b, 0, :, 1]]).astype(np.int64)
        cnt = np.bincount(u0, minlength=N_USER).astype(np.float32)
        A = np.zeros((N_USER, N_LOC), np.float32)
        np.add.at(A, (u0, l0), 1.0)
        Ahat = A / np.maximum(cnt, 1.0)[:, None]
        Mb = np.zeros((N_USER, N_LOC), np.float32)
        Tb = np.zeros((N_USER, N_LOC), np.float32)
        Mb[mob[b, 0, :, 0], mob[b, 0, :, 1]] = 1.0
        Tb[text[b, 0, :, 0], text[b, 0, :, 1]] = 1.0
        M = Mb + Tb
        has0 = (cnt > 0).astype(np.float32)
        n_with = max(float(has0.sum()), 1.0)
        nh_cnt = float(N_USER) - float(has0.sum())
        hw = has0 / n_with

        w_mwe = (hw[:, None] * Ahat).sum(0)
        w_sxu = Ahat.sum(0) + nh_cnt * w_mwe
        w_h0 = (w_sxu + 2.0) / 3072.0

        ne = np.zeros(3, np.float32)
        for dd in range(3):
            us = np.concatenate([mob[b, dd + 1, :, 0], text[b, dd + 1, :, 0]])
            hasE = np.zeros(N_USER, np.float32)
            hasE[us] = 1.0
            ne[dd] = hasE.sum()
        nec = np.zeros(6, np.float32)
        for dd in range(3):
            nec[2 * dd] = ne[dd] / 2048.0
            nec[2 * dd + 1] = (N_USER - ne[dd]) / 3072.0

        def rollu(x, axis=0):
            return np.roll(x, -rot, axis=axis)

        wcols = np.stack([w_h0, w_mwe], axis=1).astype(np.float16)
        blob = np.concatenate(
            [dev_layout(W16), dev_layout(WT16), dev_layout(acol2),
             dev_layout(wcols)], axis=1)
        m = dict(shared)
        m.update({
            "blob": np.ascontiguousarray(blob),
            "Ahat8": dev_layout(np.ascontiguousarray(rollu(Ahat, 0).T)).astype(f8),
            "MT16": dev_layout(np.ascontiguousarray(rollu(M, 0).T)).astype(f8),
            "nothas16": rollu(1.0 - has0)[None, :].astype(np.float16),
            "necol": np.tile(nec[None, :], (P, 1)).astype(np.float32),
        })
        in_maps.append(m)
    return in_maps


def kernel(**inputs):
    from concourse.bass_utils import run_bass_kernel_spmd

    if "nc" not in _CACHE:
        _CACHE["nc"] = _build_nc()
    nc = _CACHE["nc"]

    x_loc = np.ascontiguousarray(inputs["x_loc"], np.float32)
    mob = np.asarray(inputs["mob_links"])
    text = np.asarray(inputs["text_links"])

    in_maps = _host_prep(inputs["x_loc"], inputs["mob_links"],
                         inputs["text_links"], inputs["W"], inputs["a"])
    res = run_bass_kernel_spmd(nc, in_maps, core_ids=list(range(NCORES)))

    out = np.zeros((B, D, N_USER + 2 * N_LOC, HD), np.float32)
    for c in range(NCORES):
        b, r = c // 2, c % 2
        o0 = np.asarray(res.results[c]["o0"], np.float32)
        o1 = np.asarray(res.results[c]["o1"], np.float32)
        out[b, 0, r * 512:(r + 1) * 512] = (
            o0.reshape(P, 4, HD).transpose(1, 0, 2).reshape(512, HD))
        out[b, 1, r * 512:(r + 1) * 512] = (
            o1.reshape(P, 4, HD).transpose(1, 0, 2).reshape(512, HD))
        if r == 0:
            rows = np.asarray(res.results[c]["rows"], np.float32)
            out[b, 0, N_USER:N_USER + N_LOC] = x_loc
            out[b, 0, N_USER + N_LOC:] = x_loc
            v1 = np.concatenate([rows[:, 0], rows[:, 1]])
            out[b, 1, N_USER:] = v1[None, :]
            for dd, d in enumerate((2, 3, 4)):
                s = 2 + 4 * dd
                r1 = np.concatenate([rows[:, s], rows[:, s + 1]])
                vn = np.concatenate([rows[:, s + 2], rows[:, s + 3]])
                us = np.concatenate([mob[b, d - 1, :, 0], text[b, d - 1, :, 0]])
                hasE = np.zeros(N_USER, bool)
                hasE[us] = True
                out[b, d, :N_USER] = np.where(hasE[:, None], r1[None, :],
                                              vn[None, :])
                out[b, d, N_USER:] = vn[None, :]
    return out


# revision 18
# speedup vs baseline: 2.3061x; 1.0229x over previous
"""Trainium2 Bass kernel for nn_BiGNN_53772990546511.

Math (validated vs reference in mathcheck.py, global l2 rel ~2.5e-4):
  - relu(elu(x)) == relu(x).
  - Day-1 attention collapses to users x locs with multiplicity mask M.
    Softmax is invariant to per-column scaling, so
      exp(leaky(f1_u + f2_l)) ~ max(g_l, t_u) scaled per-row by c_l,
    with g = exp(0.8 f2 + B), t = exp(-0.8 f1 + B), c = exp(0.2 f2 + G).
    The c_l row factor is folded into the whext rows (including the ones
    column), so PT = max(g_l, t_u) * M needs only 2 elementwise ops/tile.
  - Days 2..4 are rank-2 row algebra kept in column form on device (no
    transposes); only 10 row-vectors leave the device and the host
    broadcasts them into the full output (pure gather/unshard).
  - All global sums over users (mwe, mean h0) are rank-1: host prepares
    index-derived weight vectors (w_mwe, w_h0) so phase-2 only computes
    the 512 own-user rows.
  - Device writes per core: 0.5MB (day0+day1 own user halves, f16) +
    rows; everything else is broadcast host-side from row vectors.

Sharding: 8 cores = 4 batches x 2 user-halves.  Odd cores get index
tensors rotated by 512 along users so the program is SPMD-uniform.
"""
import numpy as np

N_USER = 1024
N_LOC = 1024
DM = 256
HD = 256
B = 4
D = 5
E = 4096
ALPHA = 0.2
BETA = -1.0
GAMMA = -0.5
P = 128
NCORES = 8

_CACHE = {}


# --------------------------------------------------------------------------
# Workarounds for this walrus build's 1-sync-wait-per-instruction limit.
# --------------------------------------------------------------------------
def _apply_tile_patch():
    import concourse.tile as tile
    from concourse.tile_sem_assignment import tick_to_sem

    if not getattr(tile.TileContext, "_drain_patched", False):
        def _patched(self, tick_clock, wait_clock):
            nc = self.nc
            gc = tick_clock.global_clock
            for proc, sem in self.sems.allocated().items():
                t = gc[proc]
                if t and t > 0:
                    nc.sync.nop().wait_op(sem, tick_to_sem(t, proc), "sem-ge")
            nc.sync.drain()
            nc.all_engine_barrier()
            popped = nc._tile_sem_poison_stack.pop()
            assert popped is self._sem_poison
            nc.clear_and_free_semaphores(list(self.sems.allocated().values()))
            nc.all_engine_barrier()

        tile.TileContext._drain_and_barrier = _patched
        tile.TileContext._drain_patched = True

    import json as _json
    import concourse.bass_utils as _bu
    import concourse.bass2jax as _b2j

    if not getattr(_bu, "_wait_split_patched", False):
        _orig_compile = _bu.compile_bir_kernel

        def _split_waits(bir_json):
            j = _json.loads(bir_json)
            nid = [0]
            for fn in j.get("functions", []):
                for bb in fn.get("blocks", []):
                    out = []
                    for inst in bb.get("instructions", []):
                        si = inst.get("sync_info") or {}
                        ow = si.get("on_wait") or []
                        if len(ow) > 1:
                            for w in ow[:-1]:
                                nid[0] += 1
                                out.append({
                                    "debug": inst.get("debug", 0),
                                    "engine": inst.get("engine", "SP"),
                                    "ins": [],
                                    "name": f"WSPL-{nid[0]}",
                                    "opcode": "NoOp",
                                    "outs": [],
                                    "sync_info": {"on_update": [],
                                                  "on_wait": [w]},
                                })
                            si["on_wait"] = [ow[-1]]
                        out.append(inst)
                    bb["instructions"] = out
            return _json.dumps(j).encode()

        def _patched_compile(bir_json, tmpdir, neff_name="file.neff"):
            return _orig_compile(_split_waits(bir_json), tmpdir,
                                 neff_name=neff_name)

        _bu.compile_bir_kernel = _patched_compile
        _b2j.compile_bir_kernel = _patched_compile
        _bu._wait_split_patched = True


def _build_nc():
    import contextlib
    import concourse.bass as bass
    import concourse.tile as tile
    from concourse import mybir

    _apply_tile_patch()
    f32 = mybir.dt.float32
    f16 = mybir.dt.float16
    f8 = mybir.dt.float8e4
    AF = mybir.ActivationFunctionType
    OP = mybir.AluOpType

    nc = bass.Bass()

    # ---------------- DRAM tensors (device layout: [128, cols]) ----------------
    d_blob = nc.dram_tensor("blob", [P, 1044], f16, kind="ExternalInput")
    d_nothas = nc.dram_tensor("nothas16", [1, N_USER], f16, kind="ExternalInput")
    d_necol = nc.dram_tensor("necol", [P, 6], f32, kind="ExternalInput")
    d_xlocT = nc.dram_tensor("xlocT16", [P, 2 * N_LOC], f16, kind="ExternalInput")
    d_xloc = nc.dram_tensor("xloc16", [P, 8 * DM], f16, kind="ExternalInput")
    d_A8 = nc.dram_tensor("Ahat8", [P, 8 * N_USER], f8, kind="ExternalInput")
    d_MT = nc.dram_tensor("MT16", [P, 8 * N_USER], f8, kind="ExternalInput")
    d_o0 = nc.dram_tensor("o0", [P, 4 * HD], f16, kind="ExternalOutput")
    d_o1 = nc.dram_tensor("o1", [P, 4 * HD], f16, kind="ExternalOutput")
    d_rows = nc.dram_tensor("rows", [P, 14], f32, kind="ExternalOutput")

    with tile.TileContext(nc) as tc:
        with contextlib.ExitStack() as ctx:
            persist = ctx.enter_context(tc.tile_pool(name="persist", bufs=1))
            work = ctx.enter_context(tc.tile_pool(name="work", bufs=1))
            psBig = ctx.enter_context(tc.tile_pool(name="psBig", bufs=2, space="PSUM"))
            ps6 = ctx.enter_context(tc.tile_pool(name="ps6", bufs=4, space="PSUM"))
            psSm = psBig
            psCol = psBig

            # ------------- input loads, in arrival-priority order -------------
            blob = persist.tile([P, 1044], f16, name="blob")
            nc.sync.dma_start(out=blob[:], in_=d_blob[:])
            Wsb = blob[:, 0:512]
            WTsb = blob[:, 512:1024]
            acol = blob[:, 1024:1028]
            wcol = blob[:, 1028:1044]
            nothas = persist.tile([1, N_USER], f16, name="nothas")
            nc.sync.dma_start(out=nothas[:], in_=d_nothas[:])
            necol = persist.tile([P, 6], f32, name="necol")
            nc.sync.dma_start(out=necol[:], in_=d_necol[:])
            xlocT = persist.tile([P, 2 * N_LOC], f16, name="xlocT")
            nc.sync.dma_start(out=xlocT[:], in_=d_xlocT[:])
            A8 = persist.tile([P, 8 * N_USER], f8, name="A8")
            nc.sync.dma_start(out=A8[:], in_=d_A8[:])
            xloc = persist.tile([P, 8 * DM], f16, name="xloc")
            nc.sync.dma_start(out=xloc[:], in_=d_xloc[:])
            MT = persist.tile([P, 8 * N_USER], f8, name="MT")
            nc.sync.dma_start(out=MT[:, 0:4 * N_USER], in_=d_MT[:, 0:4 * N_USER])
            nc.sync.dma_start(out=MT[:, 4 * N_USER:], in_=d_MT[:, 4 * N_USER:])

            def Wk(kt):
                return Wsb[:, kt * HD:(kt + 1) * HD]

            def WTk(kt):
                return WTsb[:, kt * DM:(kt + 1) * DM]

            def xT(kt, sl):
                s = slice(kt * N_LOC + sl.start, kt * N_LOC + sl.stop)
                return xlocT[:, s]

            def xl(lt):
                return xloc[:, lt * DM:(lt + 1) * DM]

            def Ah(lt, sl):
                s = slice(lt * N_USER + sl.start, lt * N_USER + sl.stop)
                return A8[:, s]

            def Mk(lt):
                return MT[:, lt * N_USER:(lt + 1) * N_USER]

            # constants
            ones16r = persist.tile([1, P], f16, name="ones16r")
            nc.vector.memset(ones16r[:], 1.0)
            one11 = persist.tile([1, 1], f16, name="one11")
            nc.vector.memset(one11[:], 1.0)
            ones16c = persist.tile([P, 1], f16, name="ones16c")
            nc.vector.memset(ones16c[:], 1.0)
            cBETA = persist.tile([P, 1], f32, name="cBETA")
            nc.vector.memset(cBETA[:], BETA)
            cGAMMA = persist.tile([P, 1], f32, name="cGAMMA")
            nc.vector.memset(cGAMMA[:], GAMMA)

            # persistent intermediates
            wa12 = [persist.tile([P, 2], f16, name=f"wa12_{kt}") for kt in range(2)]
            xw1c = persist.tile([P, 8], f16, name="xw1c")
            xw2c = persist.tile([P, 8], f32, name="xw2c")
            g32 = persist.tile([P, 8], f32, name="g32")
            c32 = persist.tile([P, 8], f32, name="c32")
            whext = [persist.tile([P, HD + 1], f16, name=f"whext{lt}")
                     for lt in range(8)]
            mh0row = persist.tile([1, DM], f16, name="mh0row")
            mwe16 = persist.tile([1, DM], f16, name="mwe16")
            mh0c = persist.tile([P, 2], f16, name="mh0c")
            mw0e = persist.tile([1, HD + 1], f16, name="mw0e")
            t16 = persist.tile([1, N_USER], f16, name="t16")
            Tb16 = persist.tile([P, N_USER], f16, name="Tb16")
            PT = persist.tile([P, 8 * N_USER], f16, name="PT")
            day0 = persist.tile([P, 4 * HD], f16, name="day0")
            h1u = persist.tile([P, 8 * HD], f16, name="h1u")
            rows_out = persist.tile([P, 14], f32, name="rows_out")
            rcol = [persist.tile([P, 1], f32, name=f"rcol{ut}") for ut in range(8)]

            # ---------------- wa12 = [W a1 | W a2] as cols ----------------
            for kt in range(2):
                ps = psSm.tile([P, 2], f32, name="psm", tag="sm", bufs=2)
                for ht in range(2):
                    nc.tensor.matmul(ps[:], WTsb[:, ht * DM + kt * P:ht * DM + (kt + 1) * P],
                                     acol[:, ht * 2:ht * 2 + 2],
                                     start=(ht == 0), stop=(ht == 1))
                nc.scalar.activation(wa12[kt][:], ps[:], AF.Copy)

            # ---------------- xw12 cols per loc tile ----------------
            for lt in range(8):
                ps = psSm.tile([P, 2], f32, name="psm", tag="sm", bufs=2)
                for kt in range(2):
                    nc.tensor.matmul(ps[:], xT(kt, slice(lt * P, (lt + 1) * P)),
                                     wa12[kt][:], start=(kt == 0), stop=(kt == 1))
                nc.vector.tensor_copy(xw1c[:, lt:lt + 1], ps[:, 0:1])
                nc.vector.tensor_copy(xw2c[:, lt:lt + 1], ps[:, 1:2])
            nc.scalar.activation(g32[:], xw2c[:], AF.Exp, scale=0.8, bias=cBETA[:])
            nc.scalar.activation(c32[:], xw2c[:], AF.Exp, scale=0.2, bias=cGAMMA[:])

            # ---------------- f1row -> t -> Tb ----------------
            for ch in range(2):
                csl = slice(ch * 512, (ch + 1) * 512)
                pf = psBig.tile([1, 512], f32, name="pf1", tag="big")
                for lt in range(8):
                    nc.tensor.matmul(pf[:], xw1c[:, lt:lt + 1], Ah(lt, csl),
                                     start=(lt == 0), stop=(lt == 7))
                nc.scalar.activation(t16[0:1, csl], pf[:], AF.Exp,
                                     scale=-0.8, bias=cBETA[0:1, :])
                ptb = psBig.tile([P, 512], f32, name="ptb", tag="big")
                nc.tensor.matmul(ptb[:], ones16r[:], t16[0:1, csl],
                                 start=True, stop=True)
                nc.scalar.activation(Tb16[:, csl], ptb[:], AF.Copy)

            # ---------------- whext_c = c_l * [Wh_l | 1] ----------------
            for lt in range(8):
                ps = psBig.tile([P, HD], f32, name="pwh", tag="big")
                for kt in range(2):
                    nc.tensor.matmul(ps[:], xT(kt, slice(lt * P, (lt + 1) * P)),
                                     Wk(kt), start=(kt == 0), stop=(kt == 1))
                nc.scalar.activation(whext[lt][:, 0:HD], ps[:], AF.Copy,
                                     scale=c32[:, lt:lt + 1])
                nc.vector.tensor_copy(whext[lt][:, HD:HD + 1], c32[:, lt:lt + 1])

            # ---------------- mh0 / mwe rows ----------------
            pmr = psBig.tile([1, DM], f32, name="pmr", tag="big")
            for lt in range(8):
                nc.tensor.matmul(pmr[:], wcol[:, lt * 2:lt * 2 + 1], xl(lt),
                                 start=(lt == 0), stop=(lt == 7))
            nc.scalar.activation(mh0row[:], pmr[:], AF.Copy)
            pmr2 = psBig.tile([1, DM], f32, name="pmr2", tag="big")
            for lt in range(8):
                nc.tensor.matmul(pmr2[:], wcol[:, lt * 2 + 1:lt * 2 + 2], xl(lt),
                                 start=(lt == 0), stop=(lt == 7))
            nc.scalar.activation(mwe16[:], pmr2[:], AF.Copy)
            for mt in range(2):
                ps = psSm.tile([P, 1], f32, name="psm", tag="sm", bufs=2)
                nc.tensor.matmul(ps[:], mh0row[0:1, mt * P:(mt + 1) * P],
                                 one11[:], start=True, stop=True)
                nc.vector.tensor_copy(mh0c[:, mt:mt + 1], ps[:])
            pmw = psBig.tile([1, HD], f32, name="pmw", tag="big")
            for kt in range(2):
                nc.tensor.matmul(pmw[:], mh0c[:, kt:kt + 1], Wk(kt),
                                 start=(kt == 0), stop=(kt == 1))
            nc.scalar.activation(mw0e[:, 0:HD], pmw[:], AF.Copy)
            nc.vector.memset(mw0e[:, HD:HD + 1], 1.0)

            # mw0 cols -> v1 (day-1 loc row) and vs-chain init
            vs1 = [work.tile([P, 3], f16, name=f"vs1_{kt}", tag=f"vs1_{kt}")
                   for kt in range(2)]
            for mt in range(2):
                ps = psCol.tile([P, 1], f32, name="pcol", tag="sm", bufs=2)
                for kt in range(2):
                    nc.tensor.matmul(ps[:], Wsb[:, kt * HD + mt * P:kt * HD + (mt + 1) * P],
                                     mh0c[:, kt:kt + 1],
                                     start=(kt == 0), stop=(kt == 1))
                nc.scalar.activation(rows_out[:, mt:mt + 1], ps[:], AF.Relu)
                nc.scalar.activation(vs1[mt][:, 0:1], ps[:], AF.Relu,
                                     scale=2048.0 / 3072.0)

            # ---------------- PT + phase 6 (interleaved over loc tiles) -----
            ph2ps = [None] * 4

            def ph2_piece(ut, piece):
                if piece == 0:
                    ph2ps[ut] = psBig.tile([P, DM], f32, name="px", tag="big")
                    for lt2 in range(4):
                        nc.tensor.matmul(
                            ph2ps[ut][:],
                            Ah(lt2, slice(ut * P, (ut + 1) * P)), xl(lt2),
                            start=(lt2 == 0), stop=False)
                else:
                    for lt2 in range(4, 8):
                        nc.tensor.matmul(
                            ph2ps[ut][:],
                            Ah(lt2, slice(ut * P, (ut + 1) * P)), xl(lt2),
                            start=False, stop=False)
                    nc.tensor.matmul(ph2ps[ut][:],
                                     nothas[0:1, ut * P:(ut + 1) * P],
                                     mwe16[:], start=False, stop=True)
                    nc.scalar.activation(day0[:, ut * HD:(ut + 1) * HD],
                                         ph2ps[ut][:], AF.Copy)

            for half in range(2):
                tiles = [ps6.tile([P, HD + 1], f32, name=f"p6_{half}_{i}",
                                  tag="p6") for i in range(4)]
                for lt in range(8):
                    csl = slice(lt * N_USER + half * 512,
                                lt * N_USER + half * 512 + 512)
                    tsl = slice(half * 512, half * 512 + 512)
                    nc.vector.scalar_tensor_tensor(
                        out=PT[:, csl], in0=Tb16[:, tsl],
                        scalar=g32[:, lt:lt + 1], in1=MT[:, csl],
                        op0=OP.max, op1=OP.mult)
                    for i in range(4):
                        ut = half * 4 + i
                        usl = slice(lt * N_USER + ut * P,
                                    lt * N_USER + (ut + 1) * P)
                        nc.tensor.matmul(tiles[i][:], PT[:, usl], whext[lt][:],
                                         start=(lt == 0), stop=False)
                    if half == 0:
                        ph2_piece(lt // 2, lt % 2)
                for i in range(4):
                    ut = half * 4 + i
                    nc.tensor.matmul(tiles[i][:],
                                     nothas[0:1, ut * P:(ut + 1) * P],
                                     mw0e[:], start=False, stop=True)
                    nc.vector.reciprocal(rcol[ut][:], tiles[i][:, HD:HD + 1])
                    nc.scalar.activation(h1u[:, ut * HD:(ut + 1) * HD],
                                         tiles[i][:, 0:HD], AF.Relu,
                                         scale=rcol[ut][:])
                if half == 0:
                    nc.scalar.dma_start(out=d_o0[:, :], in_=day0[:])
                    nc.scalar.dma_start(out=d_o1[:, :], in_=h1u[:, 0:4 * HD])

            # ---------------- scol ----------------
            for mt in range(2):
                ps = psCol.tile([P, 1], f32, name="pcol", tag="sm", bufs=2)
                for ut in range(8):
                    nc.tensor.matmul(
                        ps[:], h1u[:, ut * HD + mt * P:ut * HD + (mt + 1) * P],
                        ones16c[:], start=(ut == 0), stop=(ut == 7))
                nc.scalar.activation(vs1[mt][:, 1:2], ps[:], AF.Copy,
                                     scale=1.0 / 3072.0)
                nc.vector.tensor_tensor(out=vs1[mt][:, 2:3], in0=vs1[mt][:, 0:1],
                                        in1=vs1[mt][:, 1:2], op=OP.add)

            # ---------------- days 2..4 ----------------
            vs_cur = vs1
            for dd in range(3):
                base = 2 + 4 * dd
                vs_n = [work.tile([P, 3], f16, name=f"vs{dd}_{kt}",
                                  tag=f"vs{dd}_{kt}") for kt in range(2)]
                for mt in range(2):
                    ps = psSm.tile([P, 3], f32, name="psm", tag="sm", bufs=2)
                    for kt in range(2):
                        nc.tensor.matmul(ps[:], Wsb[:, kt * HD + mt * P:kt * HD + (mt + 1) * P],
                                         vs_cur[kt][:],
                                         start=(kt == 0), stop=(kt == 1))
                    nc.scalar.activation(rows_out[:, base + mt:base + mt + 1],
                                         ps[:, 0:1], AF.Relu, scale=1.5)
                    nc.scalar.activation(rows_out[:, base + 2 + mt:base + 3 + mt],
                                         ps[:, 2:3], AF.Relu)
                    if dd < 2:
                        nc.scalar.activation(vs_n[mt][:, 0:1], ps[:, 2:3],
                                             AF.Relu, scale=2048.0 / 3072.0)
                        tA = work.tile([P, 1], f32, name=f"tA{dd}_{mt}",
                                       tag=f"tA{dd}_{mt}")
                        tB = work.tile([P, 1], f32, name=f"tB{dd}_{mt}",
                                       tag=f"tB{dd}_{mt}")
                        nc.scalar.activation(tA[:], ps[:, 0:1], AF.Relu,
                                             scale=necol[:, 2 * dd:2 * dd + 1])
                        nc.scalar.activation(tB[:], ps[:, 2:3], AF.Relu,
                                             scale=necol[:, 2 * dd + 1:2 * dd + 2])
                        nc.vector.tensor_tensor(out=vs_n[mt][:, 1:2], in0=tA[:],
                                                in1=tB[:], op=OP.add)
                        nc.vector.tensor_tensor(out=vs_n[mt][:, 2:3],
                                                in0=vs_n[mt][:, 0:1],
                                                in1=vs_n[mt][:, 1:2], op=OP.add)
                vs_cur = vs_n
            nc.scalar.dma_start(out=d_rows[:, :], in_=rows_out[:])

    return nc


def _host_prep(x_loc, mob_links, text_links, W, a):
    """Index-derived preprocessing -> per-core input maps."""
    import ml_dtypes
    f8 = ml_dtypes.float8_e4m3

    x_loc = np.ascontiguousarray(x_loc, np.float32)
    W = np.ascontiguousarray(W, np.float32)
    a = np.ascontiguousarray(a, np.float32)
    mob = np.asarray(mob_links)
    text = np.asarray(text_links)

    def dev_layout(x):
        t = x.shape[0] // P
        return np.concatenate([x[i * P:(i + 1) * P] for i in range(t)], axis=1)

    W16 = W.astype(np.float16)
    WT16 = np.ascontiguousarray(W.T).astype(np.float16)
    acol2 = np.concatenate([a[:HD], a[HD:]], axis=1).astype(np.float16)
    shared = {
        "xlocT16": dev_layout(np.ascontiguousarray(x_loc.T).astype(np.float16)),
        "xloc16": dev_layout(x_loc.astype(np.float16)),
    }

    in_maps = []
    for c in range(NCORES):
        b, r = c // 2, c % 2
        rot = r * 512
        u0 = np.concatenate([mob[b, 0, :, 0], text[b, 0, :, 0]]).astype(np.int64)
        l0 = np.concatenate([mob[b, 0, :, 1], text[b, 0, :, 1]]).astype(np.int64)
        cnt = np.bincount(u0, minlength=N_USER).astype(np.float32)
        A = np.zeros((N_USER, N_LOC), np.float32)
        np.add.at(A, (u0, l0), 1.0)
        Ahat = A / np.maximum(cnt, 1.0)[:, None]
        Mb = np.zeros((N_USER, N_LOC), np.float32)
        Tb = np.zeros((N_USER, N_LOC), np.float32)
        Mb[mob[b, 0, :, 0], mob[b, 0, :, 1]] = 1.0
        Tb[text[b, 0, :, 0], text[b, 0, :, 1]] = 1.0
        M = Mb + Tb
        has0 = (cnt > 0).astype(np.float32)
        n_with = max(float(has0.sum()), 1.0)
        nh_cnt = float(N_USER) - float(has0.sum())
        hw = has0 / n_with

        w_mwe = (hw[:, None] * Ahat).sum(0)
        w_sxu = Ahat.sum(0) + nh_cnt * w_mwe
        w_h0 = (w_sxu + 2.0) / 3072.0

        ne = np.zeros(3, np.float32)
        for dd in range(3):
            us = np.concatenate([mob[b, dd + 1, :, 0], text[b, dd + 1, :, 0]])
            hasE = np.zeros(N_USER, np.float32)
            hasE[us] = 1.0
            ne[dd] = hasE.sum()
        nec = np.zeros(6, np.float32)
        for dd in range(3):
            nec[2 * dd] = ne[dd] / 2048.0
            nec[2 * dd + 1] = (N_USER - ne[dd]) / 3072.0

        def rollu(x, axis=0):
            return np.roll(x, -rot, axis=axis)

        wcols = np.stack([w_h0, w_mwe], axis=1).astype(np.float16)
        blob = np.concatenate(
            [dev_layout(W16), dev_layout(WT16), dev_layout(acol2),
             dev_layout(wcols)], axis=1)
        m = dict(shared)
        m.update({
            "blob": np.ascontiguousarray(blob),
            "Ahat8": dev_layout(np.ascontiguousarray(rollu(Ahat, 0).T)).astype(f8),
            "MT16": dev_layout(np.ascontiguousarray(rollu(M, 0).T)).astype(f8),
            "nothas16": rollu(1.0 - has0)[None, :].astype(np.float16),
            "necol": np.tile(nec[None, :], (P, 1)).astype(np.float32),
        })
        in_maps.append(m)
    return in_maps


def kernel(**inputs):
    from concourse.bass_utils import run_bass_kernel_spmd

    if "nc" not in _CACHE:
        _CACHE["nc"] = _build_nc()
    nc = _CACHE["nc"]

    x_loc = np.ascontiguousarray(inputs["x_loc"], np.float32)
    mob = np.asarray(inputs["mob_links"])
    text = np.asarray(inputs["text_links"])

    in_maps = _host_prep(inputs["x_loc"], inputs["mob_links"],
                         inputs["text_links"], inputs["W"], inputs["a"])
    res = run_bass_kernel_spmd(nc, in_maps, core_ids=list(range(NCORES)))

    out = np.zeros((B, D, N_USER + 2 * N_LOC, HD), np.float32)
    for c in range(NCORES):
        b, r = c // 2, c % 2
        o0 = np.asarray(res.results[c]["o0"], np.float32)
        o1 = np.asarray(res.results[c]["o1"], np.float32)
        out[b, 0, r * 512:(r + 1) * 512] = (
            o0.reshape(P, 4, HD).transpose(1, 0, 2).reshape(512, HD))
        out[b, 1, r * 512:(r + 1) * 512] = (
            o1.reshape(P, 4, HD).transpose(1, 0, 2).reshape(512, HD))
        if r == 0:
            rows = np.asarray(res.results[c]["rows"], np.float32)
            out[b, 0, N_USER:N_USER + N_LOC] = x_loc
            out[b, 0, N_USER + N_LOC:] = x_loc
            v1 = np.concatenate([rows[:, 0], rows[:, 1]])
            out[b, 1, N_USER:] = v1[None, :]
            for dd, d in enumerate((2, 3, 4)):
                s = 2 + 4 * dd
                r1 = np.concatenate([rows[:, s], rows[:, s + 1]])
                vn = np.concatenate([rows[:, s + 2], rows[:, s + 3]])
                us = np.concatenate([mob[b, d - 1, :, 0], text[b, d - 1, :, 0]])
                hasE = np.zeros(N_USER, bool)
                hasE[us] = True
                out[b, d, :N_USER] = np.where(hasE[:, None], r1[None, :],
                                              vn[None, :])
                out[b, d, N_USER:] = vn[None, :]
    return out
